# revision 1
# baseline (speedup 1.0000x reference)
"""DCNv2 deformable PS-RoI pooling on 8 Trainium2 NeuronCores — v2.

Strategy (roi-pair data-parallel):
  * Host replicates the reference coordinate math exactly (float32) and folds
    bilinear weights, validity masking and 1/count into per-roi sparse weights.
  * Rois on the same image are greedily PAIRED by bbox overlap; each pair's
    union pixel set is loaded once (shared pixels deduped). Pair pixels are
    packed into 128-row chunks (padding only at pair granularity).
  * Per chunk ONE matmul: lhsT = A_chunk [128px, 98] (49 bin-columns for each
    roi of the pair), rhs = patch_chunk [128px, 256c], accumulating
    out = psum [98, 256] f32 over the pair's chunks. This covers both rois
    and all 256 channels in a single instruction -> ~32 matmuls/core.
  * Patch pixels arrive via grouped gpsimd.dma_gather (pixel-row gather from
    the channel-last feature map); A-matrix slices load per group so the DMA
    stream pipelines: gather(g) overlaps desc-gen(g+1), matmul(g), drains and
    the per-group output DMA.
  * PSUM drains alternate DVE / Activation so neither engine serializes.
"""
import numpy as np

f32 = np.float32
f64 = np.float64

B, C, H, W = 8, 256, 64, 64
N_ROIS, P, S = 256, 7, 4
PART = 7
NJ = P * P  # 49
NJ2 = 2 * NJ  # 98: pair column block
SCALE = f32(1.0 / 16.0)
TRANS_STD = f32(0.1)
N_CORES = 8
N_GROUPS = 7
GROUP_WEIGHTS = [0.8, 1.6, 1.5, 1.2, 0.5, 0.15, 0.1, 0.1]
OUT_ENGINES = ["sync", "scalar", "gpsimd"]
PSUM_TAGS = 4
PSUM_BUFS = 2

_prog_cache = {}


# --------------------------------------------------------------------------
# host math: exact f32 replication of the reference coordinate computation
# --------------------------------------------------------------------------
def _roi_sampling_data(rois, offset):
    rois = np.asarray(rois, dtype=f32)
    offset = np.asarray(offset, dtype=f32)
    batch = rois[:, 0].astype(np.int32)

    roi_sw = np.round(rois[:, 1]) * SCALE - f32(0.5)
    roi_sh = np.round(rois[:, 2]) * SCALE - f32(0.5)
    roi_ew = (np.round(rois[:, 3]) + f32(1.0)) * SCALE - f32(0.5)
    roi_eh = (np.round(rois[:, 4]) + f32(1.0)) * SCALE - f32(0.5)
    roi_w = np.maximum(roi_ew - roi_sw, f32(0.1))
    roi_h = np.maximum(roi_eh - roi_sh, f32(0.1))
    bin_w = roi_w / f32(P)
    bin_h = roi_h / f32(P)
    sub_w = bin_w / f32(S)
    sub_h = bin_h / f32(S)

    ph = np.arange(P, dtype=np.int32)
    pw = np.arange(P, dtype=np.int32)
    part_h = np.clip(
        np.floor(ph.astype(f32) / f32(P) * f32(PART)).astype(np.int32), 0, PART - 1
    )
    part_w = np.clip(
        np.floor(pw.astype(f32) / f32(P) * f32(PART)).astype(np.int32), 0, PART - 1
    )

    tx = offset[:, 0][:, part_h[:, None], part_w[None, :]] * TRANS_STD  # (N,7,7)
    ty = offset[:, 1][:, part_h[:, None], part_w[None, :]] * TRANS_STD

    wstart = (
        pw.astype(f32)[None, None, :] * bin_w[:, None, None]
        + roi_sw[:, None, None]
        + tx * roi_w[:, None, None]
    )
    hstart = (
        ph.astype(f32)[None, :, None] * bin_h[:, None, None]
        + roi_sh[:, None, None]
        + ty * roi_h[:, None, None]
    )

    iw = np.arange(S, dtype=f32)
    ih = np.arange(S, dtype=f32)
    wpos = (
        wstart[:, :, :, None, None]
        + iw[None, None, None, None, :] * sub_w[:, None, None, None, None]
    )
    hpos = (
        hstart[:, :, :, None, None]
        + ih[None, None, None, :, None] * sub_h[:, None, None, None, None]
    )

    valid = (
        (wpos >= f32(-0.5)) & (wpos <= f32(W) - f32(0.5))
        & (hpos >= f32(-0.5)) & (hpos <= f32(H) - f32(0.5))
    )
    wc = np.clip(wpos, f32(0.0), f32(W - 1.0))
    hc = np.clip(hpos, f32(0.0), f32(H - 1.0))

    x0 = np.floor(wc).astype(np.int32)
    x1 = np.ceil(wc).astype(np.int32)
    y0 = np.floor(hc).astype(np.int32)
    y1 = np.ceil(hc).astype(np.int32)
    dx = (wc - np.floor(wc)).astype(f64)
    dy = (hc - np.floor(hc)).astype(f64)

    cnt = valid.sum(axis=(3, 4)).astype(f32)  # (N,7,7)
    coef = np.where(cnt > 0, 1.0 / np.maximum(cnt, f32(1.0)).astype(f64), 0.0)

    w00 = (1.0 - dx) * (1.0 - dy)
    w01 = dx * (1.0 - dy)
    w10 = (1.0 - dx) * dy
    w11 = dx * dy

    return dict(
        batch=batch, valid=valid, x0=x0, x1=x1, y0=y0, y1=y1,
        w00=w00, w01=w01, w10=w10, w11=w11, coef=coef,
    )


def _roi_points(d, n):
    """All (y, x, j, w) bilinear contributions of roi n, valid-masked."""
    full = (P, P, S, S)
    v = d["valid"][n]
    if not v.any():
        return None
    jj = np.broadcast_to(
        np.arange(NJ, dtype=np.int64).reshape(P, P, 1, 1), full
    )[v]
    xs0 = np.broadcast_to(d["x0"][n], full)[v]
    xs1 = np.broadcast_to(d["x1"][n], full)[v]
    ys0 = np.broadcast_to(d["y0"][n], full)[v]
    ys1 = np.broadcast_to(d["y1"][n], full)[v]
    cf = np.broadcast_to(d["coef"][n][:, :, None, None], full)[v]
    yy = np.concatenate([ys0, ys0, ys1, ys1])
    xx = np.concatenate([xs0, xs1, xs0, xs1])
    jc = np.concatenate([jj, jj, jj, jj])
    ww = np.concatenate([
        np.broadcast_to(d["w00"][n], full)[v] * cf,
        np.broadcast_to(d["w01"][n], full)[v] * cf,
        np.broadcast_to(d["w10"][n], full)[v] * cf,
        np.broadcast_to(d["w11"][n], full)[v] * cf,
    ])
    box = (int(ys0.min()), int(ys1.max()), int(xs0.min()), int(xs1.max()))
    return yy, xx, jc, ww, box


def _build_pairs(rois, offset):
    """Pair rois (same image, max bbox overlap); per pair return
    (gidx [npix_padded], W [npix_padded, 98], (roi_a, roi_b))."""
    rois = np.asarray(rois, dtype=f32)
    d = _roi_sampling_data(rois, offset)
    pts = [_roi_points(d, n) for n in range(N_ROIS)]

    def box_of(n):
        return pts[n][4] if pts[n] is not None else None

    def npix_of(n):
        bx = box_of(n)
        if bx is None:
            return 0
        return (bx[1] - bx[0] + 1) * (bx[3] - bx[2] + 1)

    def union_npix(a, b):
        ba, bb = box_of(a), box_of(b)
        if ba is None:
            return npix_of(b)
        if bb is None:
            return npix_of(a)
        dy = min(ba[1], bb[1]) - max(ba[0], bb[0]) + 1
        dx = min(ba[3], bb[3]) - max(ba[2], bb[2]) + 1
        return npix_of(a) + npix_of(b) - max(dy, 0) * max(dx, 0)

    def chunks_of(npix):
        return max((npix + 127) // 128, 1)

    batch = d["batch"]
    pairs = []  # (roi_a, roi_b | -1)
    for b in range(B):
        idxs = [n for n in range(N_ROIS) if batch[n] == b]
        while len(idxs) >= 2:
            best = None
            for i in range(len(idxs)):
                for j in range(i + 1, len(idxs)):
                    u = union_npix(idxs[i], idxs[j])
                    if chunks_of(u) > 3:
                        continue
                    save = (chunks_of(npix_of(idxs[i]))
                            + chunks_of(npix_of(idxs[j])) - chunks_of(u))
                    key = (save, -(chunks_of(u) * 128 - u))
                    if best is None or key > best[0]:
                        best = (key, i, j)
            if best is None:
                pairs.append((idxs.pop(), -1))
                continue
            _, i, j = best
            a, c = idxs[i], idxs[j]
            idxs.pop(j)
            idxs.pop(i)
            pairs.append((a, c))
        if idxs:
            pairs.append((idxs[0], -1))

    out = []
    for ra, rb in pairs:
        members = [(ra, 0)] + ([(rb, NJ)] if rb >= 0 else [])
        boxes = [box_of(n) for n, _ in members if box_of(n) is not None]
        if not boxes:
            out.append((np.zeros(128, np.int32), np.zeros((128, NJ2), f32),
                        (ra, rb)))
            continue
        uy0 = min(bx[0] for bx in boxes)
        uy1 = max(bx[1] for bx in boxes)
        ux0 = min(bx[2] for bx in boxes)
        ux1 = max(bx[3] for bx in boxes)
        uh, uw = uy1 - uy0 + 1, ux1 - ux0 + 1
        mask = np.zeros((uh, uw), bool)
        for n, _ in members:
            bx = box_of(n)
            if bx is None:
                continue
            mask[bx[0] - uy0:bx[1] + 1 - uy0, bx[2] - ux0:bx[3] + 1 - ux0] = True
        ys, xs = np.nonzero(mask)  # row-major
        npix = len(ys)
        pos = np.full((uh, uw), -1, np.int64)
        pos[ys, xs] = np.arange(npix)
        npad = (-npix) % 128
        Wm = np.zeros((npix + npad, NJ2), f64)
        for n, cb in members:
            if pts[n] is None:
                continue
            yy, xx, jc, ww = pts[n][0], pts[n][1], pts[n][2], pts[n][3]
            lp = pos[yy - uy0, xx - ux0]
            np.add.at(Wm, (lp, jc + cb), ww)
        bidx = int(batch[ra])
        gidx = (bidx * (H * W) + (uy0 + ys) * W + (ux0 + xs)).astype(np.int32)
        gidx = np.concatenate([gidx, np.zeros(npad, np.int32)])
        out.append((gidx, Wm.astype(f32), (ra, rb)))
    return out


def _partition_pairs(pairs):
    """Rank-window deal: sort pairs by descending chunk count; slot s takes
    ranks [8s, 8s+8), one per core, so nch[s] = the rank-8s value (tight)."""
    chunks_per = np.array([len(g) // 128 for g, _, _ in pairs])
    order = np.argsort(-chunks_per, kind="stable")
    nslot = (len(pairs) + N_CORES - 1) // N_CORES
    slots = [[-1] * nslot for _ in range(N_CORES)]
    for i, p in enumerate(order):
        rnd, pos = divmod(i, N_CORES)
        slots[pos][rnd] = int(p)
    nch = [
        int(max((chunks_per[slots[k][s]] if slots[k][s] >= 0 else 1)
                for k in range(N_CORES)))
        for s in range(nslot)
    ]
    # ascending slot sizes (small slots drain early, big pairs stream late),
    # with one 1-chunk slot moved to the very end as a minimal final group
    perm = list(np.argsort(nch, kind="stable"))
    perm = perm[1:] + [perm[0]]
    slots = [[sl[i] for i in perm] for sl in slots]
    nch = tuple(nch[i] for i in perm)
    return slots, nch


# --------------------------------------------------------------------------
# device program
# --------------------------------------------------------------------------
SW = C + NJ2  # 354: per-chunk stream width (patch channels | A columns)


def _build_program(nch):
    import concourse.bacc as bacc
    import concourse.mybir as mybir
    from concourse.tile import TileContext

    nslot = len(nch)
    T = int(sum(nch))
    col0 = np.concatenate([[0], np.cumsum(nch)]).astype(int)

    # last group = the final (1-chunk) slot alone; split the rest by weights
    weights = GROUP_WEIGHTS[:N_GROUPS - 1]
    cum = np.cumsum(weights) / sum(weights)
    t_head = int(col0[nslot - 1])
    bounds = [0]
    for g in range(N_GROUPS - 2):
        target = t_head * cum[g]
        s = int(np.searchsorted(col0, target))
        s = min(max(s, bounds[-1] + 1), (nslot - 1) - (N_GROUPS - 2 - g))
        bounds.append(s)
    bounds.append(nslot - 1)
    bounds.append(nslot)

    nc = bacc.Bacc("TRN2", num_devices=N_CORES)
    dt = mybir.dt
    strm = nc.dram_tensor("strm", [128, T, SW], dt.float16, kind="ExternalInput")
    outd = nc.dram_tensor("out", [NJ2, nslot, C], dt.float16, kind="ExternalOutput")

    with TileContext(nc) as tc:
        with (
            tc.tile_pool(name="main", bufs=1) as mp,
            tc.tile_pool(name="psum", bufs=PSUM_BUFS, space="PSUM") as pp,
        ):
            st = []
            obs = []
            for g in range(N_GROUPS):
                s0, s1 = bounds[g], bounds[g + 1]
                c0, c1 = int(col0[s0]), int(col0[s1])
                t_g = mp.tile([128, c1 - c0, SW], dt.float16, tag=f"strm{g}")
                nc.sync.dma_start(out=t_g[:], in_=strm[:, c0:c1, :])
                st.append(t_g)
            for g in range(N_GROUPS):
                s0, s1 = bounds[g], bounds[g + 1]
                c0 = int(col0[s0])
                t_g = st[g]
                ob = mp.tile([128, s1 - s0, C], dt.float16, tag=f"outbuf{g}")
                obs.append(ob)
                for s in range(s0, s1):
                    ps = pp.tile([128, C], dt.float32, tag=f"ps{s % PSUM_TAGS}")
                    for t in range(nch[s]):
                        c = int(col0[s]) + t
                        nc.tensor.matmul(
                            out=ps[0:NJ2, :],
                            lhsT=t_g[:, c - c0, C:SW],
                            rhs=t_g[:, c - c0, 0:C],
                            start=(t == 0),
                            stop=(t == nch[s] - 1),
                        )
                    if s % 2 == 0:
                        nc.vector.tensor_copy(
                            out=ob[0:NJ2, s - s0, :], in_=ps[0:NJ2, :]
                        )
                    else:
                        nc.scalar.copy(out=ob[0:NJ2, s - s0, :], in_=ps[0:NJ2, :])
            eng_map = {"sync": nc.sync, "scalar": nc.scalar, "gpsimd": nc.gpsimd}
            for g in range(N_GROUPS):
                s0, s1 = bounds[g], bounds[g + 1]
                eng = eng_map[OUT_ENGINES[g % len(OUT_ENGINES)]]
                eng.dma_start(out=outd[:, s0:s1, :], in_=obs[g][0:NJ2, :, :])
    nc.compile()
    return nc, bounds


# --------------------------------------------------------------------------
# entry point
# --------------------------------------------------------------------------
def kernel(input, rois, offset):
    from concourse.bass_utils import run_bass_kernel_spmd

    input = np.asarray(input, dtype=f32)
    pairs = _build_pairs(rois, offset)

    fcl = np.ascontiguousarray(
        input.transpose(0, 2, 3, 1).astype(np.float16)
    ).reshape(B * H * W, C)

    slots, nch = _partition_pairs(pairs)
    nslot = len(nch)
    T = int(sum(nch))
    col0 = np.concatenate([[0], np.cumsum(nch)]).astype(int)

    key = nch
    if key not in _prog_cache:
        _prog_cache[key] = _build_program(nch)
    nc, bounds = _prog_cache[key]

    in_maps = []
    for k in range(N_CORES):
        logical = np.zeros(T * 128, np.int64)
        a_arr = np.zeros((T * 128, NJ2), np.float16)
        for s in range(nslot):
            p = slots[k][s]
            if p < 0:
                continue
            gidx, Wm, _ = pairs[p]
            r0 = int(col0[s]) * 128
            logical[r0:r0 + len(gidx)] = gidx
            a_arr[r0:r0 + len(gidx), :] = Wm
        # stream[p, c, :] = [ patch pixel (c*128+p) channels | A row ]
        px = fcl[logical]  # (T*128, C)
        stream = np.concatenate([px, a_arr], axis=1)  # (T*128, 354)
        stream = np.ascontiguousarray(
            stream.reshape(T, 128, SW).transpose(1, 0, 2)
        )
        in_maps.append({"strm": stream})

    res = run_bass_kernel_spmd(nc, in_maps, core_ids=list(range(N_CORES)))

    out_full = np.empty((N_ROIS, C, P, P), f32)
    for k in range(N_CORES):
        arr = res.results[k]["out"].astype(f32)  # (98, nslot, 256)
        for s in range(nslot):
            p = slots[k][s]
            if p < 0:
                continue
            ra, rb = pairs[p][2]
            out_full[ra] = arr[0:NJ, s, :].T.reshape(C, P, P)
            if rb >= 0:
                out_full[rb] = arr[NJ:NJ2, s, :].T.reshape(C, P, P)
    return out_full



# revision 11
# speedup vs baseline: 1.0062x; 1.0062x over previous
"""DCNv2 deformable PS-RoI pooling on 8 Trainium2 NeuronCores — v3.

Strategy (fp8 union-pair stream, 32-row windows):
  * Host replicates the reference coordinate math exactly (f32) and folds
    bilinear weights, validity masking and 1/count into per-roi sparse weights.
  * Rois are paired per image minimizing ceil32(|union pixel set|); exact
    pixel sets (not bounding boxes) are streamed.  128 pairs -> 16 per core.
  * Pairs are rank-matched across cores: slot i spans rows
    [a_i, a_i + L_i) of a shared row space, L_i = max over cores of the
    rank-i pair's 32-aligned pixel count.  Row windows are program
    constants, so the SPMD matmul schedule (partition-sliced lhsT/rhs) is
    identical on every core while per-core pixel data differs.
  * Stream dtype float8_e3m4 for BOTH pixels and A-weights (A pre-scaled
    x8; host divides the output back).  Measured l2 rel err ~1.4e-2.
    Per stream row: 256 px bytes + 98 weight bytes = 354 B (vs 708 f16).
  * One matmul per (window, chunk) piece: out = psum[98, 256] accumulated
    over the window's chunks; partition-offset matmuls slice the window's
    rows within each 128-row chunk (legal bases {0,32,64,96}).
  * PE p-state warm-up: a run of dummy matmuls at program start ramps the
    tensor engine to full clock before real data arrives.
  * Drains alternate DVE/Act; output DMAs ride the Pool SWDGE queue so the
    HWDGE device only serves input loads.
"""
import numpy as np
import ml_dtypes

f32 = np.float32
f64 = np.float64
E3M4 = ml_dtypes.float8_e3m4

B, C, H, W = 8, 256, 64, 64
N_ROIS, P, S = 256, 7, 4
PART = 7
NJ = P * P              # 49
NJ2 = 2 * NJ            # 98 columns per pair
SCALE = f32(1.0 / 16.0)
TRANS_STD = f32(0.1)
N_CORES = 8
A_SCALE = 8.0           # weights pre-scaled x8 (max w <= 1, e3m4 max 15.5)
SW = C + NJ2            # 354 bytes per stream row (f8)
N_PSUM_TAGS = 4
WARMUP_MM = 14

_prog_cache = {}


# --------------------------------------------------------------------------
# host math: exact f32 replication of the reference coordinate computation
# --------------------------------------------------------------------------
def _roi_sampling_data(rois, offset):
    rois = np.asarray(rois, dtype=f32)
    offset = np.asarray(offset, dtype=f32)
    batch = rois[:, 0].astype(np.int32)

    roi_sw = np.round(rois[:, 1]) * SCALE - f32(0.5)
    roi_sh = np.round(rois[:, 2]) * SCALE - f32(0.5)
    roi_ew = (np.round(rois[:, 3]) + f32(1.0)) * SCALE - f32(0.5)
    roi_eh = (np.round(rois[:, 4]) + f32(1.0)) * SCALE - f32(0.5)
    roi_w = np.maximum(roi_ew - roi_sw, f32(0.1))
    roi_h = np.maximum(roi_eh - roi_sh, f32(0.1))
    bin_w = roi_w / f32(P)
    bin_h = roi_h / f32(P)
    sub_w = bin_w / f32(S)
    sub_h = bin_h / f32(S)

    ph = np.arange(P, dtype=np.int32)
    pw = np.arange(P, dtype=np.int32)
    part_h = np.clip(
        np.floor(ph.astype(f32) / f32(P) * f32(PART)).astype(np.int32), 0, PART - 1
    )
    part_w = np.clip(
        np.floor(pw.astype(f32) / f32(P) * f32(PART)).astype(np.int32), 0, PART - 1
    )

    tx = offset[:, 0][:, part_h[:, None], part_w[None, :]] * TRANS_STD  # (N,7,7)
    ty = offset[:, 1][:, part_h[:, None], part_w[None, :]] * TRANS_STD

    wstart = (
        pw.astype(f32)[None, None, :] * bin_w[:, None, None]
        + roi_sw[:, None, None]
        + tx * roi_w[:, None, None]
    )
    hstart = (
        ph.astype(f32)[None, :, None] * bin_h[:, None, None]
        + roi_sh[:, None, None]
        + ty * roi_h[:, None, None]
    )

    iw = np.arange(S, dtype=f32)
    ih = np.arange(S, dtype=f32)
    wpos = (
        wstart[:, :, :, None, None]
        + iw[None, None, None, None, :] * sub_w[:, None, None, None, None]
    )
    hpos = (
        hstart[:, :, :, None, None]
        + ih[None, None, None, :, None] * sub_h[:, None, None, None, None]
    )

    valid = (
        (wpos >= f32(-0.5)) & (wpos <= f32(W) - f32(0.5))
        & (hpos >= f32(-0.5)) & (hpos <= f32(H) - f32(0.5))
    )
    wc = np.clip(wpos, f32(0.0), f32(W - 1.0))
    hc = np.clip(hpos, f32(0.0), f32(H - 1.0))

    x0 = np.floor(wc).astype(np.int32)
    x1 = np.ceil(wc).astype(np.int32)
    y0 = np.floor(hc).astype(np.int32)
    y1 = np.ceil(hc).astype(np.int32)
    dx = (wc - np.floor(wc)).astype(f64)
    dy = (hc - np.floor(hc)).astype(f64)

    cnt = valid.sum(axis=(3, 4)).astype(f32)  # (N,7,7)
    coef = np.where(cnt > 0, 1.0 / np.maximum(cnt, f32(1.0)).astype(f64), 0.0)

    w00 = (1.0 - dx) * (1.0 - dy)
    w01 = dx * (1.0 - dy)
    w10 = (1.0 - dx) * dy
    w11 = dx * dy

    return dict(
        batch=batch, valid=valid, x0=x0, x1=x1, y0=y0, y1=y1,
        w00=w00, w01=w01, w10=w10, w11=w11, coef=coef,
    )


def _roi_points(d, n):
    """All (y, x, j, w) bilinear contributions of roi n, valid-masked."""
    full = (P, P, S, S)
    v = d["valid"][n]
    if not v.any():
        return None
    jj = np.broadcast_to(
        np.arange(NJ, dtype=np.int64).reshape(P, P, 1, 1), full
    )[v]
    xs0 = np.broadcast_to(d["x0"][n], full)[v]
    xs1 = np.broadcast_to(d["x1"][n], full)[v]
    ys0 = np.broadcast_to(d["y0"][n], full)[v]
    ys1 = np.broadcast_to(d["y1"][n], full)[v]
    cf = np.broadcast_to(d["coef"][n][:, :, None, None], full)[v]
    yy = np.concatenate([ys0, ys0, ys1, ys1])
    xx = np.concatenate([xs0, xs1, xs0, xs1])
    jc = np.concatenate([jj, jj, jj, jj])
    ww = np.concatenate([
        np.broadcast_to(d["w00"][n], full)[v] * cf,
        np.broadcast_to(d["w01"][n], full)[v] * cf,
        np.broadcast_to(d["w10"][n], full)[v] * cf,
        np.broadcast_to(d["w11"][n], full)[v] * cf,
    ])
    return yy, xx, jc, ww


def _ceil64(x):
    """Window lengths are 64-aligned: matmul partition bases are then always
    0 or 64, the only tile positions that proved safe to mix inside one PE
    accumulation group on hardware (base-32 pieces crash the neuron
    runtime when mixed with 64/128-row pieces)."""
    return max((int(x) + 63) // 64 * 64, 64)


def _plan(rois, offset):
    """Pair rois, deal pairs to cores, fix rank-max 32-aligned row windows."""
    d = _roi_sampling_data(rois, offset)
    pts = [_roi_points(d, n) for n in range(N_ROIS)]
    batch = d["batch"]

    # pixel key includes the image index so cross-image leftover pairs keep
    # their pixels distinct
    px = []
    for n in range(N_ROIS):
        if pts[n] is None:
            px.append(frozenset())
        else:
            bn = int(batch[n])
            px.append(frozenset(
                (bn, y, x)
                for y, x in zip(pts[n][0].tolist(), pts[n][1].tolist())
            ))

    pairs = []
    leftover = []
    for b in range(B):
        idx = sorted(
            [n for n in range(N_ROIS) if batch[n] == b],
            key=lambda n: (-len(px[n]), n),
        )
        while len(idx) >= 2:
            a = idx.pop(0)
            best = min(
                range(len(idx)),
                key=lambda j: (
                    _ceil64(len(px[a] | px[idx[j]])),
                    -len(px[a] & px[idx[j]]),
                    idx[j],
                ),
            )
            pairs.append((a, idx.pop(best)))
        leftover += idx
    leftover.sort()
    while len(leftover) >= 2:
        pairs.append((leftover.pop(), leftover.pop()))
    if leftover:
        pairs.append((leftover.pop(), -1))

    plen = np.array([len(px[a] | (px[b] if b >= 0 else frozenset()))
                     for a, b in pairs])

    order = np.argsort(-plen, kind="stable")
    cores = [[] for _ in range(N_CORES)]
    tot = [0] * N_CORES
    for p in order:
        k = int(np.argmin(tot))
        cores[k].append(int(p))
        tot[k] += _ceil64(plen[p])

    nslot = max(len(c) for c in cores)
    L = [
        max((_ceil64(plen[c[i]]) if i < len(c) else 64) for c in cores)
        for i in range(nslot)
    ]
    # A window with start==64 (mod 128) and L==128 emits the piece sequence
    # [64:128] -> [0:64]: same PE row-size bucket at a different tile
    # position inside one accumulation group, which crashes the neuron
    # runtime.  Pad the previous window by 64 rows to shift the start.
    a = 0
    for i in range(nslot):
        if a % 128 == 64 and L[i] == 128:
            L[i - 1] += 64
            a += 64
        a += L[i]
    R = int(sum(L))
    T = (R + 127) // 128
    return d, pts, px, pairs, cores, L, T, nslot


def _split_groups(T):
    """Input-load chunk groups: small first for a fast pipeline start."""
    sizes = []
    want = [2, 2, 3, 3, 3, 4, 4]
    i = 0
    left = T
    while left > 0:
        s = min(want[i] if i < len(want) else 4, left)
        sizes.append(s)
        left -= s
        i += 1
    return tuple(sizes)


def _out_groups(nslot):
    """Output-store slot groups; final group stays small for a short tail."""
    if nslot <= 6:
        return tuple([nslot - 1, 1]) if nslot > 1 else (1,)
    head = nslot - 1
    base = (head + 2) // 3
    gs = []
    left = head
    while left > 0:
        s = min(base, left)
        gs.append(s)
        left -= s
    gs.append(1)
    return tuple(gs)


def _window_pieces(a, bnd):
    """Legal matmul partition slices for row window [a, bnd) per 128-chunk.

    Matmul base partitions are restricted to {0,32,64,96} with size buckets
    (base 32 -> size <= 32), so a [32,128) slice splits in two.
    """
    out = []
    for c in range(a // 128, (bnd + 127) // 128):
        r0 = max(a, c * 128) - c * 128
        r1 = min(bnd, (c + 1) * 128) - c * 128
        assert r0 in (0, 64), (a, bnd, c, r0)
        out.append((c, r0, r1))
    for (c1, p0, p1), (c2, q0, q1) in zip(out, out[1:]):
        b1, b2 = (128 if p1 - p0 > 64 else 64), (128 if q1 - q0 > 64 else 64)
        assert not (b1 == b2 and p0 != q0), ("unsafe PE tile seq", out)
    return out


# --------------------------------------------------------------------------
# device program
# --------------------------------------------------------------------------
def _build_program(key):
    import concourse.bacc as bacc
    import concourse.mybir as mybir
    from concourse.tile import TileContext

    T, L, in_groups, out_grps, nslot = key
    L = list(L)

    nc = bacc.Bacc("TRN2", num_devices=N_CORES)
    dt = mybir.dt
    strm = nc.dram_tensor("strm", [128, T * SW], dt.float8e3, kind="ExternalInput")
    outd = nc.dram_tensor("out", [NJ2, nslot * C], dt.float16, kind="ExternalOutput")

    # chunk -> (group idx, local chunk offset)
    c2g = {}
    c0 = 0
    for g, nch in enumerate(in_groups):
        for c in range(c0, c0 + nch):
            c2g[c] = (g, c - c0)
        c0 += nch

    with TileContext(nc) as tc:
        with (
            tc.tile_pool(name="main", bufs=1) as mp,
            tc.tile_pool(name="psum", bufs=1, space="PSUM") as pp,
        ):
            # PE p-state warm-up: dummy matmuls on a zeroed tile keep the
            # tensor engine busy from t~0 so real matmuls run at full clock.
            zt = mp.tile([128, SW], dt.float8e3, tag="zt")
            nc.vector.memset(zt[:], 0.0)
            wps = pp.tile([128, C], dt.float32, tag="wps")
            for _ in range(WARMUP_MM):
                nc.tensor.matmul(
                    out=wps[0:NJ2, :], lhsT=zt[:, C:SW], rhs=zt[:, 0:C],
                    start=True, stop=True,
                )

            # input loads (sync/SP queue -> HWDGE)
            st = []
            c0 = 0
            for g, nch in enumerate(in_groups):
                t_g = mp.tile([128, nch * SW], dt.float8e3, tag=f"strm{g}")
                nc.sync.dma_start(
                    out=t_g[:], in_=strm[:, c0 * SW:(c0 + nch) * SW]
                )
                st.append(t_g)
                c0 += nch

            # out buffers per store group
            obs = []
            s0 = 0
            slot2grp = {}
            for g, ns in enumerate(out_grps):
                ob = mp.tile([NJ2, ns * C], dt.float16, tag=f"ob{g}")
                obs.append(ob)
                for s in range(s0, s0 + ns):
                    slot2grp[s] = (g, s - s0)
                s0 += ns

            # matmul schedule: fixed row windows, partition-sliced pieces
            a = 0
            drained = [False] * nslot
            for i in range(nslot):
                pieces = _window_pieces(a, a + L[i])
                ps = pp.tile([128, C], dt.float32, tag=f"ps{i % N_PSUM_TAGS}")
                for k, (c, r0, r1) in enumerate(pieces):
                    g, lc = c2g[c]
                    t_g = st[g]
                    col = lc * SW
                    nc.tensor.matmul(
                        out=ps[0:NJ2, :],
                        lhsT=t_g[r0:r1, col + C:col + SW],
                        rhs=t_g[r0:r1, col:col + C],
                        start=(k == 0),
                        stop=(k == len(pieces) - 1),
                    )
                g, ls = slot2grp[i]
                if i % 2 == 0:
                    nc.vector.tensor_copy(
                        out=obs[g][:, ls * C:(ls + 1) * C], in_=ps[0:NJ2, :]
                    )
                else:
                    nc.scalar.copy(
                        out=obs[g][:, ls * C:(ls + 1) * C], in_=ps[0:NJ2, :]
                    )
                drained[i] = True
                a += L[i]

            # output stores on the Pool SWDGE queue (keeps HWDGE for loads)
            s0 = 0
            for g, ns in enumerate(out_grps):
                nc.gpsimd.dma_start(
                    out=outd[:, s0 * C:(s0 + ns) * C], in_=obs[g][:]
                )
                s0 += ns
    nc.compile()
    return nc, key


# --------------------------------------------------------------------------
# entry point
# --------------------------------------------------------------------------
def kernel(input, rois, offset):
    from concourse.bass_utils import run_bass_kernel_spmd

    input = np.asarray(input, dtype=f32)
    d, pts, px, pairs, cores, L, T, nslot = _plan(rois, offset)

    in_groups = _split_groups(T)
    out_grps = _out_groups(nslot)
    key = (T, tuple(int(x) for x in L), in_groups, out_grps, nslot)
    if key not in _prog_cache:
        _prog_cache[key] = _build_program(key)
    nc, _ = _prog_cache[key]

    # channel-last fp8 feature map, flat pixel index
    fcl8 = np.ascontiguousarray(
        input.transpose(0, 2, 3, 1).astype(E3M4)
    ).reshape(B * H * W, C)

    a_starts = np.concatenate([[0], np.cumsum(L)]).astype(int)
    R = int(a_starts[-1])
    batch = d["batch"]

    in_maps = []
    for k in range(N_CORES):
        stream = np.zeros((128, T * SW), dtype=E3M4)
        srows = np.zeros((T * 128, SW), dtype=E3M4)  # row-major scratch
        for i, p in enumerate(cores[k]):
            ra, rb = pairs[p]
            members = [(ra, 0)] + ([(rb, NJ)] if rb >= 0 else [])
            pset = sorted(px[ra] | (px[rb] if rb >= 0 else frozenset()))
            if not pset:
                continue
            r0 = int(a_starts[i])
            # pixels (pset entries are (img, y, x) triples)
            bs = np.array([t[0] for t in pset])
            ys = np.array([t[1] for t in pset])
            xs = np.array([t[2] for t in pset])
            srows[r0:r0 + len(pset), 0:C] = fcl8[bs * (H * W) + ys * W + xs]
            # A-weights (accumulate taps in f64, then quantize once)
            acc = np.zeros((len(pset), NJ2), f64)
            pos = {t: r for r, t in enumerate(pset)}
            for n, cb in members:
                if pts[n] is None:
                    continue
                bn = int(batch[n])
                yy, xx, jc, ww = pts[n]
                lp = np.array([pos[(bn, y, x)]
                               for y, x in zip(yy.tolist(), xx.tolist())])
                np.add.at(acc, (lp, jc + cb), ww * A_SCALE)
            srows[r0:r0 + len(pset), C:SW] = acc.astype(f32).astype(E3M4)
        # [T*128, SW] -> [128, T, SW] -> [128, T*SW]
        stream[:] = srows.reshape(T, 128, SW).transpose(1, 0, 2).reshape(128, T * SW)
        in_maps.append({"strm": stream})

    res = run_bass_kernel_spmd(nc, in_maps, core_ids=list(range(N_CORES)))

    out_full = np.empty((N_ROIS, C, P, P), f32)
    inv = f32(1.0 / A_SCALE)
    for k in range(N_CORES):
        arr = res.results[k]["out"].astype(f32).reshape(NJ2, nslot, C)
        for i, p in enumerate(cores[k]):
            ra, rb = pairs[p]
            out_full[ra] = (arr[0:NJ, i, :] * inv).T.reshape(C, P, P)
            if rb >= 0:
                out_full[rb] = (arr[NJ:NJ2, i, :] * inv).T.reshape(C, P, P)
    return out_full


# revision 13
# speedup vs baseline: 1.0793x; 1.0726x over previous
"""DCNv2 deformable PS-RoI pooling on 8 Trainium2 NeuronCores — v3.

Strategy (fp8 union-pair stream, 32-row windows):
  * Host replicates the reference coordinate math exactly (f32) and folds
    bilinear weights, validity masking and 1/count into per-roi sparse weights.
  * Rois are paired per image minimizing ceil32(|union pixel set|); exact
    pixel sets (not bounding boxes) are streamed.  128 pairs -> 16 per core.
  * Pairs are rank-matched across cores: slot i spans rows
    [a_i, a_i + L_i) of a shared row space, L_i = max over cores of the
    rank-i pair's 32-aligned pixel count.  Row windows are program
    constants, so the SPMD matmul schedule (partition-sliced lhsT/rhs) is
    identical on every core while per-core pixel data differs.
  * Stream dtype float8_e3m4 for BOTH pixels and A-weights (A pre-scaled
    x8; host divides the output back).  Measured l2 rel err ~1.4e-2.
    Per stream row: 256 px bytes + 98 weight bytes = 354 B (vs 708 f16).
  * One matmul per (window, chunk) piece: out = psum[98, 256] accumulated
    over the window's chunks; partition-offset matmuls slice the window's
    rows within each 128-row chunk (legal bases {0,32,64,96}).
  * PE p-state warm-up: a run of dummy matmuls at program start ramps the
    tensor engine to full clock before real data arrives.
  * Drains alternate DVE/Act; output DMAs ride the Pool SWDGE queue so the
    HWDGE device only serves input loads.
"""
import numpy as np
import ml_dtypes

f32 = np.float32
f64 = np.float64
E3M4 = ml_dtypes.float8_e3m4

B, C, H, W = 8, 256, 64, 64
N_ROIS, P, S = 256, 7, 4
PART = 7
NJ = P * P              # 49
NJ2 = 2 * NJ            # 98 columns per pair
SCALE = f32(1.0 / 16.0)
TRANS_STD = f32(0.1)
N_CORES = 8
A_SCALE = 8.0           # weights pre-scaled x8 (max w <= 1, e3m4 max 15.5)
SW = C + NJ2            # 354 bytes per stream row (f8)
N_PSUM_TAGS = 4
WARMUP_MM = 14

_prog_cache = {}


# --------------------------------------------------------------------------
# host math: exact f32 replication of the reference coordinate computation
# --------------------------------------------------------------------------
def _roi_sampling_data(rois, offset):
    rois = np.asarray(rois, dtype=f32)
    offset = np.asarray(offset, dtype=f32)
    batch = rois[:, 0].astype(np.int32)

    roi_sw = np.round(rois[:, 1]) * SCALE - f32(0.5)
    roi_sh = np.round(rois[:, 2]) * SCALE - f32(0.5)
    roi_ew = (np.round(rois[:, 3]) + f32(1.0)) * SCALE - f32(0.5)
    roi_eh = (np.round(rois[:, 4]) + f32(1.0)) * SCALE - f32(0.5)
    roi_w = np.maximum(roi_ew - roi_sw, f32(0.1))
    roi_h = np.maximum(roi_eh - roi_sh, f32(0.1))
    bin_w = roi_w / f32(P)
    bin_h = roi_h / f32(P)
    sub_w = bin_w / f32(S)
    sub_h = bin_h / f32(S)

    ph = np.arange(P, dtype=np.int32)
    pw = np.arange(P, dtype=np.int32)
    part_h = np.clip(
        np.floor(ph.astype(f32) / f32(P) * f32(PART)).astype(np.int32), 0, PART - 1
    )
    part_w = np.clip(
        np.floor(pw.astype(f32) / f32(P) * f32(PART)).astype(np.int32), 0, PART - 1
    )

    tx = offset[:, 0][:, part_h[:, None], part_w[None, :]] * TRANS_STD  # (N,7,7)
    ty = offset[:, 1][:, part_h[:, None], part_w[None, :]] * TRANS_STD

    wstart = (
        pw.astype(f32)[None, None, :] * bin_w[:, None, None]
        + roi_sw[:, None, None]
        + tx * roi_w[:, None, None]
    )
    hstart = (
        ph.astype(f32)[None, :, None] * bin_h[:, None, None]
        + roi_sh[:, None, None]
        + ty * roi_h[:, None, None]
    )

    iw = np.arange(S, dtype=f32)
    ih = np.arange(S, dtype=f32)
    wpos = (
        wstart[:, :, :, None, None]
        + iw[None, None, None, None, :] * sub_w[:, None, None, None, None]
    )
    hpos = (
        hstart[:, :, :, None, None]
        + ih[None, None, None, :, None] * sub_h[:, None, None, None, None]
    )

    valid = (
        (wpos >= f32(-0.5)) & (wpos <= f32(W) - f32(0.5))
        & (hpos >= f32(-0.5)) & (hpos <= f32(H) - f32(0.5))
    )
    wc = np.clip(wpos, f32(0.0), f32(W - 1.0))
    hc = np.clip(hpos, f32(0.0), f32(H - 1.0))

    x0 = np.floor(wc).astype(np.int32)
    x1 = np.ceil(wc).astype(np.int32)
    y0 = np.floor(hc).astype(np.int32)
    y1 = np.ceil(hc).astype(np.int32)
    dx = (wc - np.floor(wc)).astype(f64)
    dy = (hc - np.floor(hc)).astype(f64)

    cnt = valid.sum(axis=(3, 4)).astype(f32)  # (N,7,7)
    coef = np.where(cnt > 0, 1.0 / np.maximum(cnt, f32(1.0)).astype(f64), 0.0)

    w00 = (1.0 - dx) * (1.0 - dy)
    w01 = dx * (1.0 - dy)
    w10 = (1.0 - dx) * dy
    w11 = dx * dy

    return dict(
        batch=batch, valid=valid, x0=x0, x1=x1, y0=y0, y1=y1,
        w00=w00, w01=w01, w10=w10, w11=w11, coef=coef,
    )


def _roi_points(d, n):
    """All (y, x, j, w) bilinear contributions of roi n, valid-masked."""
    full = (P, P, S, S)
    v = d["valid"][n]
    if not v.any():
        return None
    jj = np.broadcast_to(
        np.arange(NJ, dtype=np.int64).reshape(P, P, 1, 1), full
    )[v]
    xs0 = np.broadcast_to(d["x0"][n], full)[v]
    xs1 = np.broadcast_to(d["x1"][n], full)[v]
    ys0 = np.broadcast_to(d["y0"][n], full)[v]
    ys1 = np.broadcast_to(d["y1"][n], full)[v]
    cf = np.broadcast_to(d["coef"][n][:, :, None, None], full)[v]
    yy = np.concatenate([ys0, ys0, ys1, ys1])
    xx = np.concatenate([xs0, xs1, xs0, xs1])
    jc = np.concatenate([jj, jj, jj, jj])
    ww = np.concatenate([
        np.broadcast_to(d["w00"][n], full)[v] * cf,
        np.broadcast_to(d["w01"][n], full)[v] * cf,
        np.broadcast_to(d["w10"][n], full)[v] * cf,
        np.broadcast_to(d["w11"][n], full)[v] * cf,
    ])
    return yy, xx, jc, ww


def _ceil64(x):
    """Window lengths are 64-aligned: matmul partition bases are then always
    0 or 64, the only tile positions that proved safe to mix inside one PE
    accumulation group on hardware (base-32 pieces crash the neuron
    runtime when mixed with 64/128-row pieces)."""
    return max((int(x) + 63) // 64 * 64, 64)


def _plan(rois, offset):
    """Pair rois, deal pairs to cores, fix rank-max 32-aligned row windows."""
    d = _roi_sampling_data(rois, offset)
    pts = [_roi_points(d, n) for n in range(N_ROIS)]
    batch = d["batch"]

    # pixel key includes the image index so cross-image leftover pairs keep
    # their pixels distinct
    px = []
    for n in range(N_ROIS):
        if pts[n] is None:
            px.append(frozenset())
        else:
            bn = int(batch[n])
            px.append(frozenset(
                (bn, y, x)
                for y, x in zip(pts[n][0].tolist(), pts[n][1].tolist())
            ))

    pairs = []
    leftover = []
    for b in range(B):
        idx = sorted(
            [n for n in range(N_ROIS) if batch[n] == b],
            key=lambda n: (-len(px[n]), n),
        )
        while len(idx) >= 2:
            a = idx.pop(0)
            best = min(
                range(len(idx)),
                key=lambda j: (
                    _ceil64(len(px[a] | px[idx[j]])),
                    -len(px[a] & px[idx[j]]),
                    idx[j],
                ),
            )
            pairs.append((a, idx.pop(best)))
        leftover += idx
    leftover.sort()
    while len(leftover) >= 2:
        pairs.append((leftover.pop(), leftover.pop()))
    if leftover:
        pairs.append((leftover.pop(), -1))

    plen = np.array([len(px[a] | (px[b] if b >= 0 else frozenset()))
                     for a, b in pairs])

    order = np.argsort(-plen, kind="stable")
    cores = [[] for _ in range(N_CORES)]
    tot = [0] * N_CORES
    for p in order:
        k = int(np.argmin(tot))
        cores[k].append(int(p))
        tot[k] += _ceil64(plen[p])

    nslot = max(len(c) for c in cores)
    L = [
        max((_ceil64(plen[c[i]]) if i < len(c) else 64) for c in cores)
        for i in range(nslot)
    ]
    # A window with start==64 (mod 128) and L==128 emits the piece sequence
    # [64:128] -> [0:64]: same PE row-size bucket at a different tile
    # position inside one accumulation group, which crashes the neuron
    # runtime.  Pad the previous window by 64 rows to shift the start.
    a = 0
    for i in range(nslot):
        if a % 128 == 64 and L[i] == 128:
            L[i - 1] += 64
            a += 64
        a += L[i]
    R = int(sum(L))
    T = (R + 127) // 128
    return d, pts, px, pairs, cores, L, T, nslot


def _split_groups(T):
    """Input-load chunk groups.  Few groups: each DMA instruction costs
    ~1.2us of SEQ+HWDGE issue overhead, so k is kept small; the first group
    is small for a fast pipeline start."""
    sizes = []
    want = [2, 5, 6, 6]
    i = 0
    left = T
    while left > 0:
        s = min(want[i] if i < len(want) else 6, left)
        sizes.append(s)
        left -= s
        i += 1
    return tuple(sizes)


def _out_groups(nslot):
    """Output-store slot groups; final group stays small for a short tail."""
    if nslot <= 3:
        return tuple([nslot - 1, 1]) if nslot > 1 else (1,)
    h1 = (nslot - 1) // 2
    h2 = nslot - 1 - h1
    return (h1, h2, 1)


def _window_pieces(a, bnd):
    """Legal matmul partition slices for row window [a, bnd) per 128-chunk.

    Matmul base partitions are restricted to {0,32,64,96} with size buckets
    (base 32 -> size <= 32), so a [32,128) slice splits in two.
    """
    out = []
    for c in range(a // 128, (bnd + 127) // 128):
        r0 = max(a, c * 128) - c * 128
        r1 = min(bnd, (c + 1) * 128) - c * 128
        assert r0 in (0, 64), (a, bnd, c, r0)
        out.append((c, r0, r1))
    for (c1, p0, p1), (c2, q0, q1) in zip(out, out[1:]):
        b1, b2 = (128 if p1 - p0 > 64 else 64), (128 if q1 - q0 > 64 else 64)
        assert not (b1 == b2 and p0 != q0), ("unsafe PE tile seq", out)
    return out


# --------------------------------------------------------------------------
# device program
# --------------------------------------------------------------------------
def _build_program(key):
    import concourse.bacc as bacc
    import concourse.mybir as mybir
    from concourse.tile import TileContext

    T, L, in_groups, out_grps, nslot = key
    L = list(L)

    nc = bacc.Bacc("TRN2", num_devices=N_CORES)
    dt = mybir.dt
    strm = nc.dram_tensor("strm", [128, T * SW], dt.float8e3, kind="ExternalInput")
    outd = nc.dram_tensor("out", [NJ2, nslot * C], dt.float16, kind="ExternalOutput")

    # chunk -> (group idx, local chunk offset)
    c2g = {}
    c0 = 0
    for g, nch in enumerate(in_groups):
        for c in range(c0, c0 + nch):
            c2g[c] = (g, c - c0)
        c0 += nch

    with TileContext(nc) as tc:
        with (
            tc.tile_pool(name="main", bufs=1) as mp,
            tc.tile_pool(name="psum", bufs=1, space="PSUM") as pp,
        ):
            # PE p-state warm-up: dummy matmuls on a zeroed tile keep the
            # tensor engine busy from t~0 so real matmuls run at full clock.
            zt = mp.tile([128, SW], dt.float8e3, tag="zt")
            nc.vector.memset(zt[:], 0.0)
            wps = pp.tile([128, C], dt.float32, tag="wps")
            for _ in range(WARMUP_MM):
                nc.tensor.matmul(
                    out=wps[0:NJ2, :], lhsT=zt[:, C:SW], rhs=zt[:, 0:C],
                    start=True, stop=True,
                )

            # input loads (sync/SP queue -> HWDGE)
            st = []
            c0 = 0
            for g, nch in enumerate(in_groups):
                t_g = mp.tile([128, nch * SW], dt.float8e3, tag=f"strm{g}")
                nc.sync.dma_start(
                    out=t_g[:], in_=strm[:, c0 * SW:(c0 + nch) * SW]
                )
                st.append(t_g)
                c0 += nch

            # out buffers per store group
            obs = []
            s0 = 0
            slot2grp = {}
            for g, ns in enumerate(out_grps):
                ob = mp.tile([NJ2, ns * C], dt.float16, tag=f"ob{g}")
                obs.append(ob)
                for s in range(s0, s0 + ns):
                    slot2grp[s] = (g, s - s0)
                s0 += ns

            # matmul schedule: fixed row windows, partition-sliced pieces
            a = 0
            drained = [False] * nslot
            for i in range(nslot):
                pieces = _window_pieces(a, a + L[i])
                ps = pp.tile([128, C], dt.float32, tag=f"ps{i % N_PSUM_TAGS}")
                for k, (c, r0, r1) in enumerate(pieces):
                    g, lc = c2g[c]
                    t_g = st[g]
                    col = lc * SW
                    nc.tensor.matmul(
                        out=ps[0:NJ2, :],
                        lhsT=t_g[r0:r1, col + C:col + SW],
                        rhs=t_g[r0:r1, col:col + C],
                        start=(k == 0),
                        stop=(k == len(pieces) - 1),
                    )
                g, ls = slot2grp[i]
                if i % 2 == 0:
                    nc.vector.tensor_copy(
                        out=obs[g][:, ls * C:(ls + 1) * C], in_=ps[0:NJ2, :]
                    )
                else:
                    nc.scalar.copy(
                        out=obs[g][:, ls * C:(ls + 1) * C], in_=ps[0:NJ2, :]
                    )
                drained[i] = True
                a += L[i]

            # output stores: HWDGE is free once the few input loads are
            # issued, so alternate the two HWDGE queues for the stores
            out_engs = [nc.scalar, nc.sync]
            s0 = 0
            for g, ns in enumerate(out_grps):
                out_engs[g % 2].dma_start(
                    out=outd[:, s0 * C:(s0 + ns) * C], in_=obs[g][:]
                )
                s0 += ns
    nc.compile()
    return nc, key


# --------------------------------------------------------------------------
# entry point
# --------------------------------------------------------------------------
def kernel(input, rois, offset):
    from concourse.bass_utils import run_bass_kernel_spmd

    input = np.asarray(input, dtype=f32)
    d, pts, px, pairs, cores, L, T, nslot = _plan(rois, offset)

    in_groups = _split_groups(T)
    out_grps = _out_groups(nslot)
    key = (T, tuple(int(x) for x in L), in_groups, out_grps, nslot)
    if key not in _prog_cache:
        _prog_cache[key] = _build_program(key)
    nc, _ = _prog_cache[key]

    # channel-last fp8 feature map, flat pixel index
    fcl8 = np.ascontiguousarray(
        input.transpose(0, 2, 3, 1).astype(E3M4)
    ).reshape(B * H * W, C)

    a_starts = np.concatenate([[0], np.cumsum(L)]).astype(int)
    R = int(a_starts[-1])
    batch = d["batch"]

    in_maps = []
    for k in range(N_CORES):
        stream = np.zeros((128, T * SW), dtype=E3M4)
        srows = np.zeros((T * 128, SW), dtype=E3M4)  # row-major scratch
        for i, p in enumerate(cores[k]):
            ra, rb = pairs[p]
            members = [(ra, 0)] + ([(rb, NJ)] if rb >= 0 else [])
            pset = sorted(px[ra] | (px[rb] if rb >= 0 else frozenset()))
            if not pset:
                continue
            r0 = int(a_starts[i])
            # pixels (pset entries are (img, y, x) triples)
            bs = np.array([t[0] for t in pset])
            ys = np.array([t[1] for t in pset])
            xs = np.array([t[2] for t in pset])
            srows[r0:r0 + len(pset), 0:C] = fcl8[bs * (H * W) + ys * W + xs]
            # A-weights (accumulate taps in f64, then quantize once)
            acc = np.zeros((len(pset), NJ2), f64)
            pos = {t: r for r, t in enumerate(pset)}
            for n, cb in members:
                if pts[n] is None:
                    continue
                bn = int(batch[n])
                yy, xx, jc, ww = pts[n]
                lp = np.array([pos[(bn, y, x)]
                               for y, x in zip(yy.tolist(), xx.tolist())])
                np.add.at(acc, (lp, jc + cb), ww * A_SCALE)
            srows[r0:r0 + len(pset), C:SW] = acc.astype(f32).astype(E3M4)
        # [T*128, SW] -> [128, T, SW] -> [128, T*SW]
        stream[:] = srows.reshape(T, 128, SW).transpose(1, 0, 2).reshape(128, T * SW)
        in_maps.append({"strm": stream})

    res = run_bass_kernel_spmd(nc, in_maps, core_ids=list(range(N_CORES)))

    out_full = np.empty((N_ROIS, C, P, P), f32)
    inv = f32(1.0 / A_SCALE)
    for k in range(N_CORES):
        arr = res.results[k]["out"].astype(f32).reshape(NJ2, nslot, C)
        for i, p in enumerate(cores[k]):
            ra, rb = pairs[p]
            out_full[ra] = (arr[0:NJ, i, :] * inv).T.reshape(C, P, P)
            if rb >= 0:
                out_full[rb] = (arr[NJ:NJ2, i, :] * inv).T.reshape(C, P, P)
    return out_full


# revision 19
# speedup vs baseline: 1.0854x; 1.0057x over previous
"""DCNv2 deformable PS-RoI pooling on 8 Trainium2 NeuronCores — v3.

Strategy (fp8 union-pair stream, 32-row windows):
  * Host replicates the reference coordinate math exactly (f32) and folds
    bilinear weights, validity masking and 1/count into per-roi sparse weights.
  * Rois are paired per image minimizing ceil32(|union pixel set|); exact
    pixel sets (not bounding boxes) are streamed.  128 pairs -> 16 per core.
  * Pairs are rank-matched across cores: slot i spans rows
    [a_i, a_i + L_i) of a shared row space, L_i = max over cores of the
    rank-i pair's 32-aligned pixel count.  Row windows are program
    constants, so the SPMD matmul schedule (partition-sliced lhsT/rhs) is
    identical on every core while per-core pixel data differs.
  * Stream dtype float8_e3m4 for BOTH pixels and A-weights (A pre-scaled
    x8; host divides the output back).  Measured l2 rel err ~1.4e-2.
    Per stream row: 256 px bytes + 98 weight bytes = 354 B (vs 708 f16).
  * One matmul per (window, chunk) piece: out = psum[98, 256] accumulated
    over the window's chunks; partition-offset matmuls slice the window's
    rows within each 128-row chunk (legal bases {0,32,64,96}).
  * PE p-state warm-up: a run of dummy matmuls at program start ramps the
    tensor engine to full clock before real data arrives.
  * Drains alternate DVE/Act; output DMAs ride the Pool SWDGE queue so the
    HWDGE device only serves input loads.
"""
import numpy as np
import ml_dtypes

f32 = np.float32
f64 = np.float64
E3M4 = ml_dtypes.float8_e3m4

B, C, H, W = 8, 256, 64, 64
N_ROIS, P, S = 256, 7, 4
PART = 7
NJ = P * P              # 49
NJ2 = 2 * NJ            # 98 columns per pair
SCALE = f32(1.0 / 16.0)
TRANS_STD = f32(0.1)
N_CORES = 8
A_SCALE = 8.0           # weights pre-scaled x8 (max w <= 1, e3m4 max 15.5)
SW = C + NJ2            # 354 bytes per stream row (f8)
N_PSUM_TAGS = 4
WARMUP_MM = 2

_prog_cache = {}


# --------------------------------------------------------------------------
# host math: exact f32 replication of the reference coordinate computation
# --------------------------------------------------------------------------
def _roi_sampling_data(rois, offset):
    rois = np.asarray(rois, dtype=f32)
    offset = np.asarray(offset, dtype=f32)
    batch = rois[:, 0].astype(np.int32)

    roi_sw = np.round(rois[:, 1]) * SCALE - f32(0.5)
    roi_sh = np.round(rois[:, 2]) * SCALE - f32(0.5)
    roi_ew = (np.round(rois[:, 3]) + f32(1.0)) * SCALE - f32(0.5)
    roi_eh = (np.round(rois[:, 4]) + f32(1.0)) * SCALE - f32(0.5)
    roi_w = np.maximum(roi_ew - roi_sw, f32(0.1))
    roi_h = np.maximum(roi_eh - roi_sh, f32(0.1))
    bin_w = roi_w / f32(P)
    bin_h = roi_h / f32(P)
    sub_w = bin_w / f32(S)
    sub_h = bin_h / f32(S)

    ph = np.arange(P, dtype=np.int32)
    pw = np.arange(P, dtype=np.int32)
    part_h = np.clip(
        np.floor(ph.astype(f32) / f32(P) * f32(PART)).astype(np.int32), 0, PART - 1
    )
    part_w = np.clip(
        np.floor(pw.astype(f32) / f32(P) * f32(PART)).astype(np.int32), 0, PART - 1
    )

    tx = offset[:, 0][:, part_h[:, None], part_w[None, :]] * TRANS_STD  # (N,7,7)
    ty = offset[:, 1][:, part_h[:, None], part_w[None, :]] * TRANS_STD

    wstart = (
        pw.astype(f32)[None, None, :] * bin_w[:, None, None]
        + roi_sw[:, None, None]
        + tx * roi_w[:, None, None]
    )
    hstart = (
        ph.astype(f32)[None, :, None] * bin_h[:, None, None]
        + roi_sh[:, None, None]
        + ty * roi_h[:, None, None]
    )

    iw = np.arange(S, dtype=f32)
    ih = np.arange(S, dtype=f32)
    wpos = (
        wstart[:, :, :, None, None]
        + iw[None, None, None, None, :] * sub_w[:, None, None, None, None]
    )
    hpos = (
        hstart[:, :, :, None, None]
        + ih[None, None, None, :, None] * sub_h[:, None, None, None, None]
    )

    valid = (
        (wpos >= f32(-0.5)) & (wpos <= f32(W) - f32(0.5))
        & (hpos >= f32(-0.5)) & (hpos <= f32(H) - f32(0.5))
    )
    wc = np.clip(wpos, f32(0.0), f32(W - 1.0))
    hc = np.clip(hpos, f32(0.0), f32(H - 1.0))

    x0 = np.floor(wc).astype(np.int32)
    x1 = np.ceil(wc).astype(np.int32)
    y0 = np.floor(hc).astype(np.int32)
    y1 = np.ceil(hc).astype(np.int32)
    dx = (wc - np.floor(wc)).astype(f64)
    dy = (hc - np.floor(hc)).astype(f64)

    cnt = valid.sum(axis=(3, 4)).astype(f32)  # (N,7,7)
    coef = np.where(cnt > 0, 1.0 / np.maximum(cnt, f32(1.0)).astype(f64), 0.0)

    w00 = (1.0 - dx) * (1.0 - dy)
    w01 = dx * (1.0 - dy)
    w10 = (1.0 - dx) * dy
    w11 = dx * dy

    return dict(
        batch=batch, valid=valid, x0=x0, x1=x1, y0=y0, y1=y1,
        w00=w00, w01=w01, w10=w10, w11=w11, coef=coef,
    )


def _roi_points(d, n):
    """All (y, x, j, w) bilinear contributions of roi n, valid-masked."""
    full = (P, P, S, S)
    v = d["valid"][n]
    if not v.any():
        return None
    jj = np.broadcast_to(
        np.arange(NJ, dtype=np.int64).reshape(P, P, 1, 1), full
    )[v]
    xs0 = np.broadcast_to(d["x0"][n], full)[v]
    xs1 = np.broadcast_to(d["x1"][n], full)[v]
    ys0 = np.broadcast_to(d["y0"][n], full)[v]
    ys1 = np.broadcast_to(d["y1"][n], full)[v]
    cf = np.broadcast_to(d["coef"][n][:, :, None, None], full)[v]
    yy = np.concatenate([ys0, ys0, ys1, ys1])
    xx = np.concatenate([xs0, xs1, xs0, xs1])
    jc = np.concatenate([jj, jj, jj, jj])
    ww = np.concatenate([
        np.broadcast_to(d["w00"][n], full)[v] * cf,
        np.broadcast_to(d["w01"][n], full)[v] * cf,
        np.broadcast_to(d["w10"][n], full)[v] * cf,
        np.broadcast_to(d["w11"][n], full)[v] * cf,
    ])
    return yy, xx, jc, ww


def _ceil64(x):
    """Window lengths are 64-aligned: matmul partition bases are then always
    0 or 64, the only tile positions that proved safe to mix inside one PE
    accumulation group on hardware (base-32 pieces crash the neuron
    runtime when mixed with 64/128-row pieces)."""
    return max((int(x) + 63) // 64 * 64, 64)


def _plan(rois, offset):
    """Pair rois, deal pairs to cores, fix rank-max 32-aligned row windows."""
    d = _roi_sampling_data(rois, offset)
    pts = [_roi_points(d, n) for n in range(N_ROIS)]
    batch = d["batch"]

    # pixel key includes the image index so cross-image leftover pairs keep
    # their pixels distinct
    px = []
    for n in range(N_ROIS):
        if pts[n] is None:
            px.append(frozenset())
        else:
            bn = int(batch[n])
            px.append(frozenset(
                (bn, y, x)
                for y, x in zip(pts[n][0].tolist(), pts[n][1].tolist())
            ))

    pairs = []
    leftover = []
    for b in range(B):
        idx = sorted(
            [n for n in range(N_ROIS) if batch[n] == b],
            key=lambda n: (-len(px[n]), n),
        )
        while len(idx) >= 2:
            a = idx.pop(0)
            best = min(
                range(len(idx)),
                key=lambda j: (
                    _ceil64(len(px[a] | px[idx[j]])),
                    -len(px[a] & px[idx[j]]),
                    idx[j],
                ),
            )
            pairs.append((a, idx.pop(best)))
        leftover += idx
    leftover.sort()
    while len(leftover) >= 2:
        pairs.append((leftover.pop(), leftover.pop()))
    if leftover:
        pairs.append((leftover.pop(), -1))

    plen = np.array([len(px[a] | (px[b] if b >= 0 else frozenset()))
                     for a, b in pairs])

    order = np.argsort(-plen, kind="stable")
    cores = [[] for _ in range(N_CORES)]
    tot = [0] * N_CORES
    for p in order:
        k = int(np.argmin(tot))
        cores[k].append(int(p))
        tot[k] += _ceil64(plen[p])

    nslot = max(len(c) for c in cores)
    L = [
        max((_ceil64(plen[c[i]]) if i < len(c) else 64) for c in cores)
        for i in range(nslot)
    ]
    # A window with start==64 (mod 128) and L==128 emits the piece sequence
    # [64:128] -> [0:64]: same PE row-size bucket at a different tile
    # position inside one accumulation group, which crashes the neuron
    # runtime.  Pad the previous window by 64 rows to shift the start.
    a = 0
    for i in range(nslot):
        if a % 128 == 64 and L[i] == 128:
            L[i - 1] += 64
            a += 64
        a += L[i]
    R = int(sum(L))
    T = (R + 127) // 128
    return d, pts, px, pairs, cores, L, T, nslot


def _split_groups(T):
    """Input-load chunk groups.  Few groups: each DMA instruction costs
    ~1.2us of SEQ+HWDGE issue overhead, so k is kept small; the first group
    is small for a fast pipeline start."""
    sizes = []
    want = [2, 7, 7, 7]
    i = 0
    left = T
    while left > 0:
        s = min(want[i] if i < len(want) else 7, left)
        sizes.append(s)
        left -= s
        i += 1
    return tuple(sizes)


def _out_groups(nslot):
    """Output-store slot groups; final group stays small for a short tail."""
    if nslot <= 3:
        return tuple([nslot - 1, 1]) if nslot > 1 else (1,)
    h1 = (nslot - 1) // 2
    h2 = nslot - 1 - h1
    return (h1, h2, 1)


def _window_pieces(a, bnd):
    """Legal matmul partition slices for row window [a, bnd) per 128-chunk.

    Matmul base partitions are restricted to {0,32,64,96} with size buckets
    (base 32 -> size <= 32), so a [32,128) slice splits in two.
    """
    out = []
    for c in range(a // 128, (bnd + 127) // 128):
        r0 = max(a, c * 128) - c * 128
        r1 = min(bnd, (c + 1) * 128) - c * 128
        assert r0 in (0, 64), (a, bnd, c, r0)
        out.append((c, r0, r1))
    for (c1, p0, p1), (c2, q0, q1) in zip(out, out[1:]):
        b1, b2 = (128 if p1 - p0 > 64 else 64), (128 if q1 - q0 > 64 else 64)
        assert not (b1 == b2 and p0 != q0), ("unsafe PE tile seq", out)
    return out


# --------------------------------------------------------------------------
# device program
# --------------------------------------------------------------------------
def _build_program(key):
    import concourse.bacc as bacc
    import concourse.mybir as mybir
    from concourse.tile import TileContext

    T, L, in_groups, out_grps, nslot = key
    L = list(L)

    nc = bacc.Bacc("TRN2", num_devices=N_CORES)
    dt = mybir.dt
    strm = nc.dram_tensor("strm", [128, T * SW], dt.float8e3, kind="ExternalInput")
    outd = nc.dram_tensor("out", [NJ2, nslot * C], dt.float16, kind="ExternalOutput")

    # chunk -> (group idx, local chunk offset)
    c2g = {}
    c0 = 0
    for g, nch in enumerate(in_groups):
        for c in range(c0, c0 + nch):
            c2g[c] = (g, c - c0)
        c0 += nch

    with TileContext(nc) as tc:
        with (
            tc.tile_pool(name="main", bufs=1) as mp,
            tc.tile_pool(name="psum", bufs=1, space="PSUM") as pp,
        ):
            # PE p-state warm-up: pe_busy_start is pinned by the FIRST PE
            # instruction and never resets, so two tiny matmuls as early as
            # possible make everything ~3us later run at full clock.  The
            # memset rides the Pool engine, which is free right after the
            # preamble.
            zt = mp.tile([128, 2], dt.float8e3, tag="zt")
            nc.gpsimd.memset(zt[:], 0.0)
            wps = pp.tile([128, C], dt.float32, tag="wps")
            for _ in range(WARMUP_MM):
                nc.tensor.matmul(
                    out=wps[0:1, 0:1], lhsT=zt[:, 0:1], rhs=zt[:, 1:2],
                    start=True, stop=True,
                )

            # input loads alternate the two HWDGE queues so SEQ issue
            # (~650ns per DMA) does not serialize ahead of the transfers
            in_engs = [nc.sync, nc.scalar]
            st = []
            c0 = 0
            for g, nch in enumerate(in_groups):
                t_g = mp.tile([128, nch * SW], dt.float8e3, tag=f"strm{g}")
                in_engs[g % 2].dma_start(
                    out=t_g[:], in_=strm[:, c0 * SW:(c0 + nch) * SW]
                )
                st.append(t_g)
                c0 += nch

            # out buffers per store group
            obs = []
            s0 = 0
            slot2grp = {}
            for g, ns in enumerate(out_grps):
                ob = mp.tile([NJ2, ns * C], dt.float16, tag=f"ob{g}")
                obs.append(ob)
                for s in range(s0, s0 + ns):
                    slot2grp[s] = (g, s - s0)
                s0 += ns

            # matmul schedule: fixed row windows, partition-sliced pieces
            a = 0
            drained = [False] * nslot
            for i in range(nslot):
                pieces = _window_pieces(a, a + L[i])
                ps = pp.tile([128, C], dt.float32, tag=f"ps{i % N_PSUM_TAGS}")
                for k, (c, r0, r1) in enumerate(pieces):
                    g, lc = c2g[c]
                    t_g = st[g]
                    col = lc * SW
                    nc.tensor.matmul(
                        out=ps[0:NJ2, :],
                        lhsT=t_g[r0:r1, col + C:col + SW],
                        rhs=t_g[r0:r1, col:col + C],
                        start=(k == 0),
                        stop=(k == len(pieces) - 1),
                    )
                g, ls = slot2grp[i]
                # GPSIMD cannot read PSUM, so drains alternate DVE/Act
                if i % 2 == 0:
                    nc.vector.tensor_copy(
                        out=obs[g][:, ls * C:(ls + 1) * C], in_=ps[0:NJ2, :]
                    )
                else:
                    nc.scalar.copy(
                        out=obs[g][:, ls * C:(ls + 1) * C], in_=ps[0:NJ2, :]
                    )
                drained[i] = True
                a += L[i]

            # output stores: a DMA's SemWait occupies its whole SEQ queue, so
            # spread the three stores over SP / Pool-SWDGE / Act; the final
            # small store rides Act right after its own drain
            out_engs = [nc.sync, nc.gpsimd, nc.scalar]
            s0 = 0
            for g, ns in enumerate(out_grps):
                out_engs[g % len(out_engs)].dma_start(
                    out=outd[:, s0 * C:(s0 + ns) * C], in_=obs[g][:]
                )
                s0 += ns
    nc.compile()
    return nc, key


# --------------------------------------------------------------------------
# entry point
# --------------------------------------------------------------------------
def kernel(input, rois, offset):
    from concourse.bass_utils import run_bass_kernel_spmd

    input = np.asarray(input, dtype=f32)
    d, pts, px, pairs, cores, L, T, nslot = _plan(rois, offset)

    in_groups = _split_groups(T)
    out_grps = _out_groups(nslot)
    key = (T, tuple(int(x) for x in L), in_groups, out_grps, nslot)
    if key not in _prog_cache:
        _prog_cache[key] = _build_program(key)
    nc, _ = _prog_cache[key]

    # channel-last fp8 feature map, flat pixel index
    fcl8 = np.ascontiguousarray(
        input.transpose(0, 2, 3, 1).astype(E3M4)
    ).reshape(B * H * W, C)

    a_starts = np.concatenate([[0], np.cumsum(L)]).astype(int)
    R = int(a_starts[-1])
    batch = d["batch"]

    in_maps = []
    for k in range(N_CORES):
        stream = np.zeros((128, T * SW), dtype=E3M4)
        srows = np.zeros((T * 128, SW), dtype=E3M4)  # row-major scratch
        for i, p in enumerate(cores[k]):
            ra, rb = pairs[p]
            members = [(ra, 0)] + ([(rb, NJ)] if rb >= 0 else [])
            pset = sorted(px[ra] | (px[rb] if rb >= 0 else frozenset()))
            if not pset:
                continue
            r0 = int(a_starts[i])
            # pixels (pset entries are (img, y, x) triples)
            bs = np.array([t[0] for t in pset])
            ys = np.array([t[1] for t in pset])
            xs = np.array([t[2] for t in pset])
            srows[r0:r0 + len(pset), 0:C] = fcl8[bs * (H * W) + ys * W + xs]
            # A-weights (accumulate taps in f64, then quantize once)
            acc = np.zeros((len(pset), NJ2), f64)
            pos = {t: r for r, t in enumerate(pset)}
            for n, cb in members:
                if pts[n] is None:
                    continue
                bn = int(batch[n])
                yy, xx, jc, ww = pts[n]
                lp = np.array([pos[(bn, y, x)]
                               for y, x in zip(yy.tolist(), xx.tolist())])
                np.add.at(acc, (lp, jc + cb), ww * A_SCALE)
            srows[r0:r0 + len(pset), C:SW] = acc.astype(f32).astype(E3M4)
        # [T*128, SW] -> [128, T, SW] -> [128, T*SW]
        stream[:] = srows.reshape(T, 128, SW).transpose(1, 0, 2).reshape(128, T * SW)
        in_maps.append({"strm": stream})

    res = run_bass_kernel_spmd(nc, in_maps, core_ids=list(range(N_CORES)))

    out_full = np.empty((N_ROIS, C, P, P), f32)
    inv = f32(1.0 / A_SCALE)
    for k in range(N_CORES):
        arr = res.results[k]["out"].astype(f32).reshape(NJ2, nslot, C)
        for i, p in enumerate(cores[k]):
            ra, rb = pairs[p]
            out_full[ra] = (arr[0:NJ, i, :] * inv).T.reshape(C, P, P)
            if rb >= 0:
                out_full[rb] = (arr[NJ:NJ2, i, :] * inv).T.reshape(C, P, P)
    return out_full


# revision 22
# speedup vs baseline: 1.1266x; 1.0380x over previous
"""DCNv2 deformable PS-RoI pooling on 8 Trainium2 NeuronCores — v3.

Strategy (fp8 union-pair stream, 32-row windows):
  * Host replicates the reference coordinate math exactly (f32) and folds
    bilinear weights, validity masking and 1/count into per-roi sparse weights.
  * Rois are paired per image minimizing ceil32(|union pixel set|); exact
    pixel sets (not bounding boxes) are streamed.  128 pairs -> 16 per core.
  * Pairs are rank-matched across cores: slot i spans rows
    [a_i, a_i + L_i) of a shared row space, L_i = max over cores of the
    rank-i pair's 32-aligned pixel count.  Row windows are program
    constants, so the SPMD matmul schedule (partition-sliced lhsT/rhs) is
    identical on every core while per-core pixel data differs.
  * Stream dtype float8_e3m4 for BOTH pixels and A-weights (A pre-scaled
    x8; host divides the output back).  Measured l2 rel err ~1.4e-2.
    Per stream row: 256 px bytes + 98 weight bytes = 354 B (vs 708 f16).
  * One matmul per (window, chunk) piece: out = psum[98, 256] accumulated
    over the window's chunks; partition-offset matmuls slice the window's
    rows within each 128-row chunk (legal bases {0,32,64,96}).
  * PE p-state warm-up: a run of dummy matmuls at program start ramps the
    tensor engine to full clock before real data arrives.
  * Drains alternate DVE/Act; output DMAs ride the Pool SWDGE queue so the
    HWDGE device only serves input loads.
"""
import numpy as np
import ml_dtypes

f32 = np.float32
f64 = np.float64
E3M4 = ml_dtypes.float8_e3m4

B, C, H, W = 8, 256, 64, 64
N_ROIS, P, S = 256, 7, 4
PART = 7
NJ = P * P              # 49
NJ2 = 2 * NJ            # 98 columns per pair
SCALE = f32(1.0 / 16.0)
TRANS_STD = f32(0.1)
N_CORES = 8
A_SCALE = 8.0           # weights pre-scaled x8 (max w <= 1, e3m4 max 15.5)
SW = C + NJ2            # 354 bytes per stream row (f8)
N_PSUM_TAGS = 4
WARMUP_MM = 2

_prog_cache = {}


# --------------------------------------------------------------------------
# host math: exact f32 replication of the reference coordinate computation
# --------------------------------------------------------------------------
def _roi_sampling_data(rois, offset):
    rois = np.asarray(rois, dtype=f32)
    offset = np.asarray(offset, dtype=f32)
    batch = rois[:, 0].astype(np.int32)

    roi_sw = np.round(rois[:, 1]) * SCALE - f32(0.5)
    roi_sh = np.round(rois[:, 2]) * SCALE - f32(0.5)
    roi_ew = (np.round(rois[:, 3]) + f32(1.0)) * SCALE - f32(0.5)
    roi_eh = (np.round(rois[:, 4]) + f32(1.0)) * SCALE - f32(0.5)
    roi_w = np.maximum(roi_ew - roi_sw, f32(0.1))
    roi_h = np.maximum(roi_eh - roi_sh, f32(0.1))
    bin_w = roi_w / f32(P)
    bin_h = roi_h / f32(P)
    sub_w = bin_w / f32(S)
    sub_h = bin_h / f32(S)

    ph = np.arange(P, dtype=np.int32)
    pw = np.arange(P, dtype=np.int32)
    part_h = np.clip(
        np.floor(ph.astype(f32) / f32(P) * f32(PART)).astype(np.int32), 0, PART - 1
    )
    part_w = np.clip(
        np.floor(pw.astype(f32) / f32(P) * f32(PART)).astype(np.int32), 0, PART - 1
    )

    tx = offset[:, 0][:, part_h[:, None], part_w[None, :]] * TRANS_STD  # (N,7,7)
    ty = offset[:, 1][:, part_h[:, None], part_w[None, :]] * TRANS_STD

    wstart = (
        pw.astype(f32)[None, None, :] * bin_w[:, None, None]
        + roi_sw[:, None, None]
        + tx * roi_w[:, None, None]
    )
    hstart = (
        ph.astype(f32)[None, :, None] * bin_h[:, None, None]
        + roi_sh[:, None, None]
        + ty * roi_h[:, None, None]
    )

    iw = np.arange(S, dtype=f32)
    ih = np.arange(S, dtype=f32)
    wpos = (
        wstart[:, :, :, None, None]
        + iw[None, None, None, None, :] * sub_w[:, None, None, None, None]
    )
    hpos = (
        hstart[:, :, :, None, None]
        + ih[None, None, None, :, None] * sub_h[:, None, None, None, None]
    )

    valid = (
        (wpos >= f32(-0.5)) & (wpos <= f32(W) - f32(0.5))
        & (hpos >= f32(-0.5)) & (hpos <= f32(H) - f32(0.5))
    )
    wc = np.clip(wpos, f32(0.0), f32(W - 1.0))
    hc = np.clip(hpos, f32(0.0), f32(H - 1.0))

    x0 = np.floor(wc).astype(np.int32)
    x1 = np.ceil(wc).astype(np.int32)
    y0 = np.floor(hc).astype(np.int32)
    y1 = np.ceil(hc).astype(np.int32)
    dx = (wc - np.floor(wc)).astype(f64)
    dy = (hc - np.floor(hc)).astype(f64)

    cnt = valid.sum(axis=(3, 4)).astype(f32)  # (N,7,7)
    coef = np.where(cnt > 0, 1.0 / np.maximum(cnt, f32(1.0)).astype(f64), 0.0)

    w00 = (1.0 - dx) * (1.0 - dy)
    w01 = dx * (1.0 - dy)
    w10 = (1.0 - dx) * dy
    w11 = dx * dy

    return dict(
        batch=batch, valid=valid, x0=x0, x1=x1, y0=y0, y1=y1,
        w00=w00, w01=w01, w10=w10, w11=w11, coef=coef,
    )


def _roi_points(d, n):
    """All (y, x, j, w) bilinear contributions of roi n, valid-masked."""
    full = (P, P, S, S)
    v = d["valid"][n]
    if not v.any():
        return None
    jj = np.broadcast_to(
        np.arange(NJ, dtype=np.int64).reshape(P, P, 1, 1), full
    )[v]
    xs0 = np.broadcast_to(d["x0"][n], full)[v]
    xs1 = np.broadcast_to(d["x1"][n], full)[v]
    ys0 = np.broadcast_to(d["y0"][n], full)[v]
    ys1 = np.broadcast_to(d["y1"][n], full)[v]
    cf = np.broadcast_to(d["coef"][n][:, :, None, None], full)[v]
    yy = np.concatenate([ys0, ys0, ys1, ys1])
    xx = np.concatenate([xs0, xs1, xs0, xs1])
    jc = np.concatenate([jj, jj, jj, jj])
    ww = np.concatenate([
        np.broadcast_to(d["w00"][n], full)[v] * cf,
        np.broadcast_to(d["w01"][n], full)[v] * cf,
        np.broadcast_to(d["w10"][n], full)[v] * cf,
        np.broadcast_to(d["w11"][n], full)[v] * cf,
    ])
    return yy, xx, jc, ww


def _ceil64(x):
    """Window lengths are 64-aligned: matmul partition bases are then always
    0 or 64, the only tile positions that proved safe to mix inside one PE
    accumulation group on hardware (base-32 pieces crash the neuron
    runtime when mixed with 64/128-row pieces)."""
    return max((int(x) + 63) // 64 * 64, 64)


def _plan(rois, offset):
    """Pair rois, deal pairs to cores, fix rank-max 32-aligned row windows."""
    d = _roi_sampling_data(rois, offset)
    pts = [_roi_points(d, n) for n in range(N_ROIS)]
    batch = d["batch"]

    # pixel key includes the image index so cross-image leftover pairs keep
    # their pixels distinct
    px = []
    for n in range(N_ROIS):
        if pts[n] is None:
            px.append(frozenset())
        else:
            bn = int(batch[n])
            px.append(frozenset(
                (bn, y, x)
                for y, x in zip(pts[n][0].tolist(), pts[n][1].tolist())
            ))

    pairs = []
    leftover = []
    for b in range(B):
        idx = sorted(
            [n for n in range(N_ROIS) if batch[n] == b],
            key=lambda n: (-len(px[n]), n),
        )
        while len(idx) >= 2:
            a = idx.pop(0)
            best = min(
                range(len(idx)),
                key=lambda j: (
                    _ceil64(len(px[a] | px[idx[j]])),
                    -len(px[a] & px[idx[j]]),
                    idx[j],
                ),
            )
            pairs.append((a, idx.pop(best)))
        leftover += idx
    leftover.sort()
    while len(leftover) >= 2:
        pairs.append((leftover.pop(), leftover.pop()))
    if leftover:
        pairs.append((leftover.pop(), -1))

    plen = np.array([len(px[a] | (px[b] if b >= 0 else frozenset()))
                     for a, b in pairs])

    order = np.argsort(-plen, kind="stable")
    cores = [[] for _ in range(N_CORES)]
    tot = [0] * N_CORES
    for p in order:
        k = int(np.argmin(tot))
        cores[k].append(int(p))
        tot[k] += _ceil64(plen[p])
    # ascending window order: small windows early (feed early output
    # groups), the single biggest window last so only ~3 matmul pieces and
    # one drain trail the final input transfer
    for k in range(N_CORES):
        cores[k] = cores[k][::-1]

    nslot = max(len(c) for c in cores)
    L = [
        max((_ceil64(plen[c[i]]) if i < len(c) else 64) for c in cores)
        for i in range(nslot)
    ]
    # A window with start==64 (mod 128) and L==128 emits the piece sequence
    # [64:128] -> [0:64]: same PE row-size bucket at a different tile
    # position inside one accumulation group, which crashes the neuron
    # runtime.  Pad the previous window by 64 rows to shift the start.
    a = 0
    for i in range(nslot):
        if a % 128 == 64 and L[i] == 128:
            L[i - 1] += 64
            a += 64
        a += L[i]
    R = int(sum(L))
    T = (R + 127) // 128
    return d, pts, px, pairs, cores, L, T, nslot


def _split_groups(T):
    """Input-load chunk groups.  Few groups: each DMA instruction costs
    ~1.2us of SEQ+HWDGE issue overhead, so k is kept small; the first group
    is small for a fast pipeline start."""
    sizes = []
    want = [2, 7, 7, 7]
    i = 0
    left = T
    while left > 0:
        s = min(want[i] if i < len(want) else 7, left)
        sizes.append(s)
        left -= s
        i += 1
    return tuple(sizes)


def _out_groups(nslot):
    """Output-store slot groups; final group stays small for a short tail."""
    if nslot <= 3:
        return tuple([nslot - 1, 1]) if nslot > 1 else (1,)
    h1 = (nslot + 1) // 2
    h2 = nslot - 1 - h1
    return (h1, h2, 1)


def _window_pieces(a, bnd):
    """Legal matmul partition slices for row window [a, bnd) per 128-chunk.

    Matmul base partitions are restricted to {0,32,64,96} with size buckets
    (base 32 -> size <= 32), so a [32,128) slice splits in two.
    """
    out = []
    for c in range(a // 128, (bnd + 127) // 128):
        r0 = max(a, c * 128) - c * 128
        r1 = min(bnd, (c + 1) * 128) - c * 128
        assert r0 in (0, 64), (a, bnd, c, r0)
        out.append((c, r0, r1))
    for (c1, p0, p1), (c2, q0, q1) in zip(out, out[1:]):
        b1, b2 = (128 if p1 - p0 > 64 else 64), (128 if q1 - q0 > 64 else 64)
        assert not (b1 == b2 and p0 != q0), ("unsafe PE tile seq", out)
    return out


# --------------------------------------------------------------------------
# device program
# --------------------------------------------------------------------------
def _build_program(key):
    import concourse.bacc as bacc
    import concourse.mybir as mybir
    from concourse.tile import TileContext

    T, L, in_groups, out_grps, nslot = key
    L = list(L)

    nc = bacc.Bacc("TRN2", num_devices=N_CORES)
    dt = mybir.dt
    strm = nc.dram_tensor("strm", [128, T * SW], dt.float8e3, kind="ExternalInput")
    outd = nc.dram_tensor("out", [NJ2, nslot * C], dt.float16, kind="ExternalOutput")

    # chunk -> (group idx, local chunk offset)
    c2g = {}
    c0 = 0
    for g, nch in enumerate(in_groups):
        for c in range(c0, c0 + nch):
            c2g[c] = (g, c - c0)
        c0 += nch

    with TileContext(nc) as tc:
        with (
            tc.tile_pool(name="main", bufs=1) as mp,
            tc.tile_pool(name="psum", bufs=1, space="PSUM") as pp,
        ):
            # PE p-state warm-up: pe_busy_start is pinned by the FIRST PE
            # instruction and never resets, so two tiny matmuls as early as
            # possible make everything ~3us later run at full clock.  The
            # memset rides the Pool engine, which is free right after the
            # preamble.
            zt = mp.tile([128, 2], dt.float8e3, tag="zt")
            nc.gpsimd.memset(zt[:], 0.0)
            wps = pp.tile([128, C], dt.float32, tag="wps")
            for _ in range(WARMUP_MM):
                nc.tensor.matmul(
                    out=wps[0:1, 0:1], lhsT=zt[:, 0:1], rhs=zt[:, 1:2],
                    start=True, stop=True,
                )

            # input loads alternate the two HWDGE queues so SEQ issue
            # (~650ns per DMA) does not serialize ahead of the transfers
            in_engs = [nc.sync, nc.scalar]
            st = []
            c0 = 0
            for g, nch in enumerate(in_groups):
                t_g = mp.tile([128, nch * SW], dt.float8e3, tag=f"strm{g}")
                in_engs[g % 2].dma_start(
                    out=t_g[:], in_=strm[:, c0 * SW:(c0 + nch) * SW]
                )
                st.append(t_g)
                c0 += nch

            # out buffers per store group
            obs = []
            s0 = 0
            slot2grp = {}
            for g, ns in enumerate(out_grps):
                ob = mp.tile([NJ2, ns * C], dt.float16, tag=f"ob{g}")
                obs.append(ob)
                for s in range(s0, s0 + ns):
                    slot2grp[s] = (g, s - s0)
                s0 += ns

            # matmul schedule: fixed row windows, partition-sliced pieces.
            # Each output store is emitted right after its group's last drain
            # so SEQ-queue ordering never couples it to later windows.
            out_engs = [nc.sync, nc.scalar, nc.sync]
            grp_end = {}
            s0 = 0
            for g, ns in enumerate(out_grps):
                grp_end[s0 + ns - 1] = g
                s0 += ns
            gs0 = np.concatenate([[0], np.cumsum(out_grps)]).astype(int)
            a = 0
            for i in range(nslot):
                pieces = _window_pieces(a, a + L[i])
                ps = pp.tile([128, C], dt.float32, tag=f"ps{i % N_PSUM_TAGS}")
                for k, (c, r0, r1) in enumerate(pieces):
                    g, lc = c2g[c]
                    t_g = st[g]
                    col = lc * SW
                    nc.tensor.matmul(
                        out=ps[0:NJ2, :],
                        lhsT=t_g[r0:r1, col + C:col + SW],
                        rhs=t_g[r0:r1, col:col + C],
                        start=(k == 0),
                        stop=(k == len(pieces) - 1),
                    )
                g, ls = slot2grp[i]
                # GPSIMD cannot read PSUM, so drains alternate DVE/Act
                if i % 2 == 0:
                    nc.vector.tensor_copy(
                        out=obs[g][:, ls * C:(ls + 1) * C], in_=ps[0:NJ2, :]
                    )
                else:
                    nc.scalar.copy(
                        out=obs[g][:, ls * C:(ls + 1) * C], in_=ps[0:NJ2, :]
                    )
                if i in grp_end:
                    g = grp_end[i]
                    out_engs[g % len(out_engs)].dma_start(
                        out=outd[:, int(gs0[g]) * C:int(gs0[g + 1]) * C],
                        in_=obs[g][:],
                    )
                a += L[i]
    nc.compile()
    return nc, key


# --------------------------------------------------------------------------
# entry point
# --------------------------------------------------------------------------
def kernel(input, rois, offset):
    from concourse.bass_utils import run_bass_kernel_spmd

    input = np.asarray(input, dtype=f32)
    d, pts, px, pairs, cores, L, T, nslot = _plan(rois, offset)

    in_groups = _split_groups(T)
    out_grps = _out_groups(nslot)
    key = (T, tuple(int(x) for x in L), in_groups, out_grps, nslot)
    if key not in _prog_cache:
        _prog_cache[key] = _build_program(key)
    nc, _ = _prog_cache[key]

    # channel-last fp8 feature map, flat pixel index
    fcl8 = np.ascontiguousarray(
        input.transpose(0, 2, 3, 1).astype(E3M4)
    ).reshape(B * H * W, C)

    a_starts = np.concatenate([[0], np.cumsum(L)]).astype(int)
    R = int(a_starts[-1])
    batch = d["batch"]

    in_maps = []
    for k in range(N_CORES):
        stream = np.zeros((128, T * SW), dtype=E3M4)
        srows = np.zeros((T * 128, SW), dtype=E3M4)  # row-major scratch
        for i, p in enumerate(cores[k]):
            ra, rb = pairs[p]
            members = [(ra, 0)] + ([(rb, NJ)] if rb >= 0 else [])
            pset = sorted(px[ra] | (px[rb] if rb >= 0 else frozenset()))
            if not pset:
                continue
            r0 = int(a_starts[i])
            # pixels (pset entries are (img, y, x) triples)
            bs = np.array([t[0] for t in pset])
            ys = np.array([t[1] for t in pset])
            xs = np.array([t[2] for t in pset])
            srows[r0:r0 + len(pset), 0:C] = fcl8[bs * (H * W) + ys * W + xs]
            # A-weights (accumulate taps in f64, then quantize once)
            acc = np.zeros((len(pset), NJ2), f64)
            pos = {t: r for r, t in enumerate(pset)}
            for n, cb in members:
                if pts[n] is None:
                    continue
                bn = int(batch[n])
                yy, xx, jc, ww = pts[n]
                lp = np.array([pos[(bn, y, x)]
                               for y, x in zip(yy.tolist(), xx.tolist())])
                np.add.at(acc, (lp, jc + cb), ww * A_SCALE)
            srows[r0:r0 + len(pset), C:SW] = acc.astype(f32).astype(E3M4)
        # [T*128, SW] -> [128, T, SW] -> [128, T*SW]
        stream[:] = srows.reshape(T, 128, SW).transpose(1, 0, 2).reshape(128, T * SW)
        in_maps.append({"strm": stream})

    res = run_bass_kernel_spmd(nc, in_maps, core_ids=list(range(N_CORES)))

    out_full = np.empty((N_ROIS, C, P, P), f32)
    inv = f32(1.0 / A_SCALE)
    for k in range(N_CORES):
        arr = res.results[k]["out"].astype(f32).reshape(NJ2, nslot, C)
        for i, p in enumerate(cores[k]):
            ra, rb = pairs[p]
            out_full[ra] = (arr[0:NJ, i, :] * inv).T.reshape(C, P, P)
            if rb >= 0:
                out_full[rb] = (arr[NJ:NJ2, i, :] * inv).T.reshape(C, P, P)
    return out_full


# revision 23
# speedup vs baseline: 1.1513x; 1.0220x over previous
"""DCNv2 deformable PS-RoI pooling on 8 Trainium2 NeuronCores — v3.

Strategy (fp8 union-pair stream, 32-row windows):
  * Host replicates the reference coordinate math exactly (f32) and folds
    bilinear weights, validity masking and 1/count into per-roi sparse weights.
  * Rois are paired per image minimizing ceil32(|union pixel set|); exact
    pixel sets (not bounding boxes) are streamed.  128 pairs -> 16 per core.
  * Pairs are rank-matched across cores: slot i spans rows
    [a_i, a_i + L_i) of a shared row space, L_i = max over cores of the
    rank-i pair's 32-aligned pixel count.  Row windows are program
    constants, so the SPMD matmul schedule (partition-sliced lhsT/rhs) is
    identical on every core while per-core pixel data differs.
  * Stream dtype float8_e3m4 for BOTH pixels and A-weights (A pre-scaled
    x8; host divides the output back).  Measured l2 rel err ~1.4e-2.
    Per stream row: 256 px bytes + 98 weight bytes = 354 B (vs 708 f16).
  * One matmul per (window, chunk) piece: out = psum[98, 256] accumulated
    over the window's chunks; partition-offset matmuls slice the window's
    rows within each 128-row chunk (legal bases {0,32,64,96}).
  * PE p-state warm-up: a run of dummy matmuls at program start ramps the
    tensor engine to full clock before real data arrives.
  * Drains alternate DVE/Act; output DMAs ride the Pool SWDGE queue so the
    HWDGE device only serves input loads.
"""
import numpy as np
import ml_dtypes

f32 = np.float32
f64 = np.float64
E3M4 = ml_dtypes.float8_e3m4

B, C, H, W = 8, 256, 64, 64
N_ROIS, P, S = 256, 7, 4
PART = 7
NJ = P * P              # 49
NJ2 = 2 * NJ            # 98 columns per pair
SCALE = f32(1.0 / 16.0)
TRANS_STD = f32(0.1)
N_CORES = 8
A_SCALE = 8.0           # weights pre-scaled x8 (max w <= 1, e3m4 max 15.5)
SW = C + NJ2            # 354 bytes per stream row (f8)
N_PSUM_TAGS = 7
WARMUP_MM = 2

_prog_cache = {}


# --------------------------------------------------------------------------
# host math: exact f32 replication of the reference coordinate computation
# --------------------------------------------------------------------------
def _roi_sampling_data(rois, offset):
    rois = np.asarray(rois, dtype=f32)
    offset = np.asarray(offset, dtype=f32)
    batch = rois[:, 0].astype(np.int32)

    roi_sw = np.round(rois[:, 1]) * SCALE - f32(0.5)
    roi_sh = np.round(rois[:, 2]) * SCALE - f32(0.5)
    roi_ew = (np.round(rois[:, 3]) + f32(1.0)) * SCALE - f32(0.5)
    roi_eh = (np.round(rois[:, 4]) + f32(1.0)) * SCALE - f32(0.5)
    roi_w = np.maximum(roi_ew - roi_sw, f32(0.1))
    roi_h = np.maximum(roi_eh - roi_sh, f32(0.1))
    bin_w = roi_w / f32(P)
    bin_h = roi_h / f32(P)
    sub_w = bin_w / f32(S)
    sub_h = bin_h / f32(S)

    ph = np.arange(P, dtype=np.int32)
    pw = np.arange(P, dtype=np.int32)
    part_h = np.clip(
        np.floor(ph.astype(f32) / f32(P) * f32(PART)).astype(np.int32), 0, PART - 1
    )
    part_w = np.clip(
        np.floor(pw.astype(f32) / f32(P) * f32(PART)).astype(np.int32), 0, PART - 1
    )

    tx = offset[:, 0][:, part_h[:, None], part_w[None, :]] * TRANS_STD  # (N,7,7)
    ty = offset[:, 1][:, part_h[:, None], part_w[None, :]] * TRANS_STD

    wstart = (
        pw.astype(f32)[None, None, :] * bin_w[:, None, None]
        + roi_sw[:, None, None]
        + tx * roi_w[:, None, None]
    )
    hstart = (
        ph.astype(f32)[None, :, None] * bin_h[:, None, None]
        + roi_sh[:, None, None]
        + ty * roi_h[:, None, None]
    )

    iw = np.arange(S, dtype=f32)
    ih = np.arange(S, dtype=f32)
    wpos = (
        wstart[:, :, :, None, None]
        + iw[None, None, None, None, :] * sub_w[:, None, None, None, None]
    )
    hpos = (
        hstart[:, :, :, None, None]
        + ih[None, None, None, :, None] * sub_h[:, None, None, None, None]
    )

    valid = (
        (wpos >= f32(-0.5)) & (wpos <= f32(W) - f32(0.5))
        & (hpos >= f32(-0.5)) & (hpos <= f32(H) - f32(0.5))
    )
    wc = np.clip(wpos, f32(0.0), f32(W - 1.0))
    hc = np.clip(hpos, f32(0.0), f32(H - 1.0))

    x0 = np.floor(wc).astype(np.int32)
    x1 = np.ceil(wc).astype(np.int32)
    y0 = np.floor(hc).astype(np.int32)
    y1 = np.ceil(hc).astype(np.int32)
    dx = (wc - np.floor(wc)).astype(f64)
    dy = (hc - np.floor(hc)).astype(f64)

    cnt = valid.sum(axis=(3, 4)).astype(f32)  # (N,7,7)
    coef = np.where(cnt > 0, 1.0 / np.maximum(cnt, f32(1.0)).astype(f64), 0.0)

    w00 = (1.0 - dx) * (1.0 - dy)
    w01 = dx * (1.0 - dy)
    w10 = (1.0 - dx) * dy
    w11 = dx * dy

    return dict(
        batch=batch, valid=valid, x0=x0, x1=x1, y0=y0, y1=y1,
        w00=w00, w01=w01, w10=w10, w11=w11, coef=coef,
    )


def _roi_points(d, n):
    """All (y, x, j, w) bilinear contributions of roi n, valid-masked."""
    full = (P, P, S, S)
    v = d["valid"][n]
    if not v.any():
        return None
    jj = np.broadcast_to(
        np.arange(NJ, dtype=np.int64).reshape(P, P, 1, 1), full
    )[v]
    xs0 = np.broadcast_to(d["x0"][n], full)[v]
    xs1 = np.broadcast_to(d["x1"][n], full)[v]
    ys0 = np.broadcast_to(d["y0"][n], full)[v]
    ys1 = np.broadcast_to(d["y1"][n], full)[v]
    cf = np.broadcast_to(d["coef"][n][:, :, None, None], full)[v]
    yy = np.concatenate([ys0, ys0, ys1, ys1])
    xx = np.concatenate([xs0, xs1, xs0, xs1])
    jc = np.concatenate([jj, jj, jj, jj])
    ww = np.concatenate([
        np.broadcast_to(d["w00"][n], full)[v] * cf,
        np.broadcast_to(d["w01"][n], full)[v] * cf,
        np.broadcast_to(d["w10"][n], full)[v] * cf,
        np.broadcast_to(d["w11"][n], full)[v] * cf,
    ])
    return yy, xx, jc, ww


def _ceil64(x):
    """Window lengths are 64-aligned: matmul partition bases are then always
    0 or 64, the only tile positions that proved safe to mix inside one PE
    accumulation group on hardware (base-32 pieces crash the neuron
    runtime when mixed with 64/128-row pieces)."""
    return max((int(x) + 63) // 64 * 64, 64)


def _plan(rois, offset):
    """Pair rois, deal pairs to cores, fix rank-max 32-aligned row windows."""
    d = _roi_sampling_data(rois, offset)
    pts = [_roi_points(d, n) for n in range(N_ROIS)]
    batch = d["batch"]

    # pixel key includes the image index so cross-image leftover pairs keep
    # their pixels distinct
    px = []
    for n in range(N_ROIS):
        if pts[n] is None:
            px.append(frozenset())
        else:
            bn = int(batch[n])
            px.append(frozenset(
                (bn, y, x)
                for y, x in zip(pts[n][0].tolist(), pts[n][1].tolist())
            ))

    pairs = []
    leftover = []
    for b in range(B):
        idx = sorted(
            [n for n in range(N_ROIS) if batch[n] == b],
            key=lambda n: (-len(px[n]), n),
        )
        while len(idx) >= 2:
            a = idx.pop(0)
            best = min(
                range(len(idx)),
                key=lambda j: (
                    _ceil64(len(px[a] | px[idx[j]])),
                    -len(px[a] & px[idx[j]]),
                    idx[j],
                ),
            )
            pairs.append((a, idx.pop(best)))
        leftover += idx
    leftover.sort()
    while len(leftover) >= 2:
        pairs.append((leftover.pop(), leftover.pop()))
    if leftover:
        pairs.append((leftover.pop(), -1))

    plen = np.array([len(px[a] | (px[b] if b >= 0 else frozenset()))
                     for a, b in pairs])

    order = np.argsort(-plen, kind="stable")
    cores = [[] for _ in range(N_CORES)]
    tot = [0] * N_CORES
    for p in order:
        k = int(np.argmin(tot))
        cores[k].append(int(p))
        tot[k] += _ceil64(plen[p])
    # ascending window order: small windows early (feed early output
    # groups), the single biggest window last so only ~3 matmul pieces and
    # one drain trail the final input transfer
    for k in range(N_CORES):
        cores[k] = cores[k][::-1]

    nslot = max(len(c) for c in cores)
    L = [
        max((_ceil64(plen[c[i]]) if i < len(c) else 64) for c in cores)
        for i in range(nslot)
    ]
    # A window with start==64 (mod 128) and L==128 emits the piece sequence
    # [64:128] -> [0:64]: same PE row-size bucket at a different tile
    # position inside one accumulation group, which crashes the neuron
    # runtime.  Pad the previous window by 64 rows to shift the start.
    a = 0
    for i in range(nslot):
        if a % 128 == 64 and L[i] == 128:
            L[i - 1] += 64
            a += 64
        a += L[i]
    R = int(sum(L))
    T = (R + 127) // 128
    return d, pts, px, pairs, cores, L, T, nslot


def _split_groups(T):
    """Input-load chunk groups.  Few groups: each DMA instruction costs
    ~1.2us of SEQ+HWDGE issue overhead, so k is kept small; the first group
    is small for a fast pipeline start."""
    sizes = []
    want = [2, 7, 7, 5]
    i = 0
    left = T
    while left > 0:
        s = min(want[i] if i < len(want) else 2, left)
        sizes.append(s)
        left -= s
        i += 1
    return tuple(sizes)


def _out_groups(nslot):
    """Output-store slot groups; final group stays small for a short tail."""
    if nslot <= 3:
        return tuple([nslot - 1, 1]) if nslot > 1 else (1,)
    h1 = max(nslot - 9, 1)
    rest = nslot - h1 - 1
    h2 = (rest + 1) // 2
    h3 = rest - h2
    return tuple(x for x in (h1, h2, h3, 1) if x > 0)


def _window_pieces(a, bnd):
    """Legal matmul partition slices for row window [a, bnd) per 128-chunk.

    Matmul base partitions are restricted to {0,32,64,96} with size buckets
    (base 32 -> size <= 32), so a [32,128) slice splits in two.
    """
    out = []
    for c in range(a // 128, (bnd + 127) // 128):
        r0 = max(a, c * 128) - c * 128
        r1 = min(bnd, (c + 1) * 128) - c * 128
        assert r0 in (0, 64), (a, bnd, c, r0)
        out.append((c, r0, r1))
    for (c1, p0, p1), (c2, q0, q1) in zip(out, out[1:]):
        b1, b2 = (128 if p1 - p0 > 64 else 64), (128 if q1 - q0 > 64 else 64)
        assert not (b1 == b2 and p0 != q0), ("unsafe PE tile seq", out)
    return out


# --------------------------------------------------------------------------
# device program
# --------------------------------------------------------------------------
def _build_program(key):
    import concourse.bacc as bacc
    import concourse.mybir as mybir
    from concourse.tile import TileContext

    T, L, in_groups, out_grps, nslot = key
    L = list(L)

    nc = bacc.Bacc("TRN2", num_devices=N_CORES)
    dt = mybir.dt
    strm = nc.dram_tensor("strm", [128, T * SW], dt.float8e3, kind="ExternalInput")
    outd = nc.dram_tensor("out", [NJ2, nslot * C], dt.float16, kind="ExternalOutput")

    # chunk -> (group idx, local chunk offset)
    c2g = {}
    c0 = 0
    for g, nch in enumerate(in_groups):
        for c in range(c0, c0 + nch):
            c2g[c] = (g, c - c0)
        c0 += nch

    with TileContext(nc) as tc:
        with (
            tc.tile_pool(name="main", bufs=1) as mp,
            tc.tile_pool(name="psum", bufs=1, space="PSUM") as pp,
        ):
            # PE p-state warm-up: pe_busy_start is pinned by the FIRST PE
            # instruction and never resets, so two tiny matmuls as early as
            # possible make everything ~3us later run at full clock.  The
            # memset rides the Pool engine, which is free right after the
            # preamble.
            zt = mp.tile([128, 2], dt.float8e3, tag="zt")
            nc.gpsimd.memset(zt[:], 0.0)
            wps = pp.tile([128, C], dt.float32, tag="wps")
            for _ in range(WARMUP_MM):
                nc.tensor.matmul(
                    out=wps[0:1, 0:1], lhsT=zt[:, 0:1], rhs=zt[:, 1:2],
                    start=True, stop=True,
                )

            # input loads alternate the two HWDGE queues so SEQ issue
            # (~650ns per DMA) does not serialize ahead of the transfers
            in_engs = [nc.sync, nc.scalar]
            st = []
            c0 = 0
            for g, nch in enumerate(in_groups):
                t_g = mp.tile([128, nch * SW], dt.float8e3, tag=f"strm{g}")
                in_engs[g % 2].dma_start(
                    out=t_g[:], in_=strm[:, c0 * SW:(c0 + nch) * SW]
                )
                st.append(t_g)
                c0 += nch

            # out buffers per store group
            obs = []
            s0 = 0
            slot2grp = {}
            for g, ns in enumerate(out_grps):
                ob = mp.tile([NJ2, ns * C], dt.float16, tag=f"ob{g}")
                obs.append(ob)
                for s in range(s0, s0 + ns):
                    slot2grp[s] = (g, s - s0)
                s0 += ns

            # matmul schedule: fixed row windows, partition-sliced pieces.
            # Each output store is emitted right after its group's last drain
            # so SEQ-queue ordering never couples it to later windows.
            out_engs = [nc.sync, nc.scalar, nc.gpsimd, nc.sync]
            grp_end = {}
            s0 = 0
            for g, ns in enumerate(out_grps):
                grp_end[s0 + ns - 1] = g
                s0 += ns
            gs0 = np.concatenate([[0], np.cumsum(out_grps)]).astype(int)
            a = 0
            for i in range(nslot):
                pieces = _window_pieces(a, a + L[i])
                ps = pp.tile([128, C], dt.float32, tag=f"ps{i % N_PSUM_TAGS}")
                for k, (c, r0, r1) in enumerate(pieces):
                    g, lc = c2g[c]
                    t_g = st[g]
                    col = lc * SW
                    nc.tensor.matmul(
                        out=ps[0:NJ2, :],
                        lhsT=t_g[r0:r1, col + C:col + SW],
                        rhs=t_g[r0:r1, col:col + C],
                        start=(k == 0),
                        stop=(k == len(pieces) - 1),
                    )
                g, ls = slot2grp[i]
                # GPSIMD cannot read PSUM, so drains alternate DVE/Act
                if i % 2 == 0:
                    nc.vector.tensor_copy(
                        out=obs[g][:, ls * C:(ls + 1) * C], in_=ps[0:NJ2, :]
                    )
                else:
                    nc.scalar.copy(
                        out=obs[g][:, ls * C:(ls + 1) * C], in_=ps[0:NJ2, :]
                    )
                if i in grp_end:
                    g = grp_end[i]
                    out_engs[g % len(out_engs)].dma_start(
                        out=outd[:, int(gs0[g]) * C:int(gs0[g + 1]) * C],
                        in_=obs[g][:],
                    )
                a += L[i]
    nc.compile()
    return nc, key


# --------------------------------------------------------------------------
# entry point
# --------------------------------------------------------------------------
def kernel(input, rois, offset):
    from concourse.bass_utils import run_bass_kernel_spmd

    input = np.asarray(input, dtype=f32)
    d, pts, px, pairs, cores, L, T, nslot = _plan(rois, offset)

    in_groups = _split_groups(T)
    out_grps = _out_groups(nslot)
    key = (T, tuple(int(x) for x in L), in_groups, out_grps, nslot)
    if key not in _prog_cache:
        _prog_cache[key] = _build_program(key)
    nc, _ = _prog_cache[key]

    # channel-last fp8 feature map, flat pixel index
    fcl8 = np.ascontiguousarray(
        input.transpose(0, 2, 3, 1).astype(E3M4)
    ).reshape(B * H * W, C)

    a_starts = np.concatenate([[0], np.cumsum(L)]).astype(int)
    R = int(a_starts[-1])
    batch = d["batch"]

    in_maps = []
    for k in range(N_CORES):
        stream = np.zeros((128, T * SW), dtype=E3M4)
        srows = np.zeros((T * 128, SW), dtype=E3M4)  # row-major scratch
        for i, p in enumerate(cores[k]):
            ra, rb = pairs[p]
            members = [(ra, 0)] + ([(rb, NJ)] if rb >= 0 else [])
            pset = sorted(px[ra] | (px[rb] if rb >= 0 else frozenset()))
            if not pset:
                continue
            r0 = int(a_starts[i])
            # pixels (pset entries are (img, y, x) triples)
            bs = np.array([t[0] for t in pset])
            ys = np.array([t[1] for t in pset])
            xs = np.array([t[2] for t in pset])
            srows[r0:r0 + len(pset), 0:C] = fcl8[bs * (H * W) + ys * W + xs]
            # A-weights (accumulate taps in f64, then quantize once)
            acc = np.zeros((len(pset), NJ2), f64)
            pos = {t: r for r, t in enumerate(pset)}
            for n, cb in members:
                if pts[n] is None:
                    continue
                bn = int(batch[n])
                yy, xx, jc, ww = pts[n]
                lp = np.array([pos[(bn, y, x)]
                               for y, x in zip(yy.tolist(), xx.tolist())])
                np.add.at(acc, (lp, jc + cb), ww * A_SCALE)
            srows[r0:r0 + len(pset), C:SW] = acc.astype(f32).astype(E3M4)
        # [T*128, SW] -> [128, T, SW] -> [128, T*SW]
        stream[:] = srows.reshape(T, 128, SW).transpose(1, 0, 2).reshape(128, T * SW)
        in_maps.append({"strm": stream})

    res = run_bass_kernel_spmd(nc, in_maps, core_ids=list(range(N_CORES)))

    out_full = np.empty((N_ROIS, C, P, P), f32)
    inv = f32(1.0 / A_SCALE)
    for k in range(N_CORES):
        arr = res.results[k]["out"].astype(f32).reshape(NJ2, nslot, C)
        for i, p in enumerate(cores[k]):
            ra, rb = pairs[p]
            out_full[ra] = (arr[0:NJ, i, :] * inv).T.reshape(C, P, P)
            if rb >= 0:
                out_full[rb] = (arr[NJ:NJ2, i, :] * inv).T.reshape(C, P, P)
    return out_full


# revision 24
# speedup vs baseline: 1.1918x; 1.0352x over previous
"""DCNv2 deformable PS-RoI pooling on 8 Trainium2 NeuronCores — v3.

Strategy (fp8 union-pair stream, 32-row windows):
  * Host replicates the reference coordinate math exactly (f32) and folds
    bilinear weights, validity masking and 1/count into per-roi sparse weights.
  * Rois are paired per image minimizing ceil32(|union pixel set|); exact
    pixel sets (not bounding boxes) are streamed.  128 pairs -> 16 per core.
  * Pairs are rank-matched across cores: slot i spans rows
    [a_i, a_i + L_i) of a shared row space, L_i = max over cores of the
    rank-i pair's 32-aligned pixel count.  Row windows are program
    constants, so the SPMD matmul schedule (partition-sliced lhsT/rhs) is
    identical on every core while per-core pixel data differs.
  * Stream dtype float8_e3m4 for BOTH pixels and A-weights (A pre-scaled
    x8; host divides the output back).  Measured l2 rel err ~1.4e-2.
    Per stream row: 256 px bytes + 98 weight bytes = 354 B (vs 708 f16).
  * One matmul per (window, chunk) piece: out = psum[98, 256] accumulated
    over the window's chunks; partition-offset matmuls slice the window's
    rows within each 128-row chunk (legal bases {0,32,64,96}).
  * PE p-state warm-up: a run of dummy matmuls at program start ramps the
    tensor engine to full clock before real data arrives.
  * Drains alternate DVE/Act; output DMAs ride the Pool SWDGE queue so the
    HWDGE device only serves input loads.
"""
import numpy as np
import ml_dtypes

f32 = np.float32
f64 = np.float64
E3M4 = ml_dtypes.float8_e3m4

B, C, H, W = 8, 256, 64, 64
N_ROIS, P, S = 256, 7, 4
PART = 7
NJ = P * P              # 49
NJ2 = 2 * NJ            # 98 columns per pair
SCALE = f32(1.0 / 16.0)
TRANS_STD = f32(0.1)
N_CORES = 8
A_SCALE = 8.0           # weights pre-scaled x8 (max w <= 1, e3m4 max 15.5)
SW = C + NJ2            # 354 bytes per stream row (f8)
N_PSUM_TAGS = 7
WARMUP_MM = 2

_prog_cache = {}


# --------------------------------------------------------------------------
# host math: exact f32 replication of the reference coordinate computation
# --------------------------------------------------------------------------
def _roi_sampling_data(rois, offset):
    rois = np.asarray(rois, dtype=f32)
    offset = np.asarray(offset, dtype=f32)
    batch = rois[:, 0].astype(np.int32)

    roi_sw = np.round(rois[:, 1]) * SCALE - f32(0.5)
    roi_sh = np.round(rois[:, 2]) * SCALE - f32(0.5)
    roi_ew = (np.round(rois[:, 3]) + f32(1.0)) * SCALE - f32(0.5)
    roi_eh = (np.round(rois[:, 4]) + f32(1.0)) * SCALE - f32(0.5)
    roi_w = np.maximum(roi_ew - roi_sw, f32(0.1))
    roi_h = np.maximum(roi_eh - roi_sh, f32(0.1))
    bin_w = roi_w / f32(P)
    bin_h = roi_h / f32(P)
    sub_w = bin_w / f32(S)
    sub_h = bin_h / f32(S)

    ph = np.arange(P, dtype=np.int32)
    pw = np.arange(P, dtype=np.int32)
    part_h = np.clip(
        np.floor(ph.astype(f32) / f32(P) * f32(PART)).astype(np.int32), 0, PART - 1
    )
    part_w = np.clip(
        np.floor(pw.astype(f32) / f32(P) * f32(PART)).astype(np.int32), 0, PART - 1
    )

    tx = offset[:, 0][:, part_h[:, None], part_w[None, :]] * TRANS_STD  # (N,7,7)
    ty = offset[:, 1][:, part_h[:, None], part_w[None, :]] * TRANS_STD

    wstart = (
        pw.astype(f32)[None, None, :] * bin_w[:, None, None]
        + roi_sw[:, None, None]
        + tx * roi_w[:, None, None]
    )
    hstart = (
        ph.astype(f32)[None, :, None] * bin_h[:, None, None]
        + roi_sh[:, None, None]
        + ty * roi_h[:, None, None]
    )

    iw = np.arange(S, dtype=f32)
    ih = np.arange(S, dtype=f32)
    wpos = (
        wstart[:, :, :, None, None]
        + iw[None, None, None, None, :] * sub_w[:, None, None, None, None]
    )
    hpos = (
        hstart[:, :, :, None, None]
        + ih[None, None, None, :, None] * sub_h[:, None, None, None, None]
    )

    valid = (
        (wpos >= f32(-0.5)) & (wpos <= f32(W) - f32(0.5))
        & (hpos >= f32(-0.5)) & (hpos <= f32(H) - f32(0.5))
    )
    wc = np.clip(wpos, f32(0.0), f32(W - 1.0))
    hc = np.clip(hpos, f32(0.0), f32(H - 1.0))

    x0 = np.floor(wc).astype(np.int32)
    x1 = np.ceil(wc).astype(np.int32)
    y0 = np.floor(hc).astype(np.int32)
    y1 = np.ceil(hc).astype(np.int32)
    dx = (wc - np.floor(wc)).astype(f64)
    dy = (hc - np.floor(hc)).astype(f64)

    cnt = valid.sum(axis=(3, 4)).astype(f32)  # (N,7,7)
    coef = np.where(cnt > 0, 1.0 / np.maximum(cnt, f32(1.0)).astype(f64), 0.0)

    w00 = (1.0 - dx) * (1.0 - dy)
    w01 = dx * (1.0 - dy)
    w10 = (1.0 - dx) * dy
    w11 = dx * dy

    return dict(
        batch=batch, valid=valid, x0=x0, x1=x1, y0=y0, y1=y1,
        w00=w00, w01=w01, w10=w10, w11=w11, coef=coef,
    )


def _roi_points(d, n):
    """All (y, x, j, w) bilinear contributions of roi n, valid-masked."""
    full = (P, P, S, S)
    v = d["valid"][n]
    if not v.any():
        return None
    jj = np.broadcast_to(
        np.arange(NJ, dtype=np.int64).reshape(P, P, 1, 1), full
    )[v]
    xs0 = np.broadcast_to(d["x0"][n], full)[v]
    xs1 = np.broadcast_to(d["x1"][n], full)[v]
    ys0 = np.broadcast_to(d["y0"][n], full)[v]
    ys1 = np.broadcast_to(d["y1"][n], full)[v]
    cf = np.broadcast_to(d["coef"][n][:, :, None, None], full)[v]
    yy = np.concatenate([ys0, ys0, ys1, ys1])
    xx = np.concatenate([xs0, xs1, xs0, xs1])
    jc = np.concatenate([jj, jj, jj, jj])
    ww = np.concatenate([
        np.broadcast_to(d["w00"][n], full)[v] * cf,
        np.broadcast_to(d["w01"][n], full)[v] * cf,
        np.broadcast_to(d["w10"][n], full)[v] * cf,
        np.broadcast_to(d["w11"][n], full)[v] * cf,
    ])
    return yy, xx, jc, ww


def _ceil64(x):
    """Window lengths are 64-aligned: matmul partition bases are then always
    0 or 64, the only tile positions that proved safe to mix inside one PE
    accumulation group on hardware (base-32 pieces crash the neuron
    runtime when mixed with 64/128-row pieces)."""
    return max((int(x) + 63) // 64 * 64, 64)


def _plan(rois, offset):
    """Pair rois, deal pairs to cores, fix rank-max 32-aligned row windows."""
    d = _roi_sampling_data(rois, offset)
    pts = [_roi_points(d, n) for n in range(N_ROIS)]
    batch = d["batch"]

    # pixel key includes the image index so cross-image leftover pairs keep
    # their pixels distinct
    px = []
    for n in range(N_ROIS):
        if pts[n] is None:
            px.append(frozenset())
        else:
            bn = int(batch[n])
            px.append(frozenset(
                (bn, y, x)
                for y, x in zip(pts[n][0].tolist(), pts[n][1].tolist())
            ))

    pairs = []
    leftover = []
    for b in range(B):
        idx = sorted(
            [n for n in range(N_ROIS) if batch[n] == b],
            key=lambda n: (-len(px[n]), n),
        )
        while len(idx) >= 2:
            a = idx.pop(0)
            best = min(
                range(len(idx)),
                key=lambda j: (
                    _ceil64(len(px[a] | px[idx[j]])),
                    -len(px[a] & px[idx[j]]),
                    idx[j],
                ),
            )
            pairs.append((a, idx.pop(best)))
        leftover += idx
    leftover.sort()
    while len(leftover) >= 2:
        pairs.append((leftover.pop(), leftover.pop()))
    if leftover:
        pairs.append((leftover.pop(), -1))

    plen = np.array([len(px[a] | (px[b] if b >= 0 else frozenset()))
                     for a, b in pairs])

    order = np.argsort(-plen, kind="stable")
    cores = [[] for _ in range(N_CORES)]
    tot = [0] * N_CORES
    for p in order:
        k = int(np.argmin(tot))
        cores[k].append(int(p))
        tot[k] += _ceil64(plen[p])
    # ascending window order: small windows early (feed early output
    # groups), then one SMALL window moved to the very end so only one
    # matmul piece and one drain trail the final input transfer
    for k in range(N_CORES):
        lst = cores[k][::-1]
        cores[k] = lst[1:] + [lst[0]]

    nslot = max(len(c) for c in cores)
    L = [
        max((_ceil64(plen[c[i]]) if i < len(c) else 64) for c in cores)
        for i in range(nslot)
    ]
    # A window with start==64 (mod 128) and L==128 emits the piece sequence
    # [64:128] -> [0:64]: same PE row-size bucket at a different tile
    # position inside one accumulation group, which crashes the neuron
    # runtime.  Pad the previous window by 64 rows to shift the start.
    a = 0
    for i in range(nslot):
        if a % 128 == 64 and L[i] == 128:
            L[i - 1] += 64
            a += 64
        a += L[i]
    R = int(sum(L))
    T = (R + 127) // 128
    return d, pts, px, pairs, cores, L, T, nslot


def _split_groups(T):
    """Input-load chunk groups.  Few groups: each DMA instruction costs
    ~1.2us of SEQ+HWDGE issue overhead, so k is kept small; the first group
    is small for a fast pipeline start."""
    assert T >= 4
    mid = T - 3
    m1 = (mid + 1) // 2
    return (2, m1, mid - m1, 1)


def _out_groups(nslot):
    """Output-store slot groups; final group stays small for a short tail."""
    if nslot <= 3:
        return tuple([nslot - 1, 1]) if nslot > 1 else (1,)
    h1 = max(nslot - 9, 1)
    rest = nslot - h1 - 1
    h2 = (rest + 1) // 2
    h3 = rest - h2
    return tuple(x for x in (h1, h2, h3, 1) if x > 0)


def _window_pieces(a, bnd):
    """Legal matmul partition slices for row window [a, bnd) per 128-chunk.

    Matmul base partitions are restricted to {0,32,64,96} with size buckets
    (base 32 -> size <= 32), so a [32,128) slice splits in two.
    """
    out = []
    for c in range(a // 128, (bnd + 127) // 128):
        r0 = max(a, c * 128) - c * 128
        r1 = min(bnd, (c + 1) * 128) - c * 128
        assert r0 in (0, 64), (a, bnd, c, r0)
        out.append((c, r0, r1))
    for (c1, p0, p1), (c2, q0, q1) in zip(out, out[1:]):
        b1, b2 = (128 if p1 - p0 > 64 else 64), (128 if q1 - q0 > 64 else 64)
        assert not (b1 == b2 and p0 != q0), ("unsafe PE tile seq", out)
    return out


# --------------------------------------------------------------------------
# device program
# --------------------------------------------------------------------------
def _build_program(key):
    import concourse.bacc as bacc
    import concourse.mybir as mybir
    from concourse.tile import TileContext

    T, L, in_groups, out_grps, nslot = key
    L = list(L)

    nc = bacc.Bacc("TRN2", num_devices=N_CORES)
    dt = mybir.dt
    strm = nc.dram_tensor("strm", [128, T * SW], dt.float8e3, kind="ExternalInput")
    outd = nc.dram_tensor("out", [NJ2, nslot * C], dt.float16, kind="ExternalOutput")

    # chunk -> (group idx, local chunk offset)
    c2g = {}
    c0 = 0
    for g, nch in enumerate(in_groups):
        for c in range(c0, c0 + nch):
            c2g[c] = (g, c - c0)
        c0 += nch

    with TileContext(nc) as tc:
        with (
            tc.tile_pool(name="main", bufs=1) as mp,
            tc.tile_pool(name="psum", bufs=1, space="PSUM") as pp,
        ):
            # PE p-state warm-up: pe_busy_start is pinned by the FIRST PE
            # instruction and never resets, so two tiny matmuls as early as
            # possible make everything ~3us later run at full clock.  The
            # memset rides the Pool engine, which is free right after the
            # preamble.
            zt = mp.tile([128, 2], dt.float8e3, tag="zt")
            nc.gpsimd.memset(zt[:], 0.0)
            wps = pp.tile([128, C], dt.float32, tag="wps")
            for _ in range(WARMUP_MM):
                nc.tensor.matmul(
                    out=wps[0:1, 0:1], lhsT=zt[:, 0:1], rhs=zt[:, 1:2],
                    start=True, stop=True,
                )

            # input loads alternate the two HWDGE queues so SEQ issue
            # (~650ns per DMA) does not serialize ahead of the transfers
            in_engs = [nc.sync, nc.scalar]
            st = []
            c0 = 0
            for g, nch in enumerate(in_groups):
                t_g = mp.tile([128, nch * SW], dt.float8e3, tag=f"strm{g}")
                in_engs[g % 2].dma_start(
                    out=t_g[:], in_=strm[:, c0 * SW:(c0 + nch) * SW]
                )
                st.append(t_g)
                c0 += nch

            # out buffers per store group
            obs = []
            s0 = 0
            slot2grp = {}
            for g, ns in enumerate(out_grps):
                ob = mp.tile([NJ2, ns * C], dt.float16, tag=f"ob{g}")
                obs.append(ob)
                for s in range(s0, s0 + ns):
                    slot2grp[s] = (g, s - s0)
                s0 += ns

            # matmul schedule: fixed row windows, partition-sliced pieces.
            # Each output store is emitted right after its group's last drain
            # so SEQ-queue ordering never couples it to later windows.
            out_engs = [nc.sync, nc.gpsimd, nc.sync, nc.gpsimd]
            grp_end = {}
            s0 = 0
            for g, ns in enumerate(out_grps):
                grp_end[s0 + ns - 1] = g
                s0 += ns
            gs0 = np.concatenate([[0], np.cumsum(out_grps)]).astype(int)
            a = 0
            for i in range(nslot):
                pieces = _window_pieces(a, a + L[i])
                ps = pp.tile([128, C], dt.float32, tag=f"ps{i % N_PSUM_TAGS}")
                for k, (c, r0, r1) in enumerate(pieces):
                    g, lc = c2g[c]
                    t_g = st[g]
                    col = lc * SW
                    nc.tensor.matmul(
                        out=ps[0:NJ2, :],
                        lhsT=t_g[r0:r1, col + C:col + SW],
                        rhs=t_g[r0:r1, col:col + C],
                        start=(k == 0),
                        stop=(k == len(pieces) - 1),
                    )
                g, ls = slot2grp[i]
                # GPSIMD cannot read PSUM, so drains alternate DVE/Act
                if i % 2 == 0:
                    nc.vector.tensor_copy(
                        out=obs[g][:, ls * C:(ls + 1) * C], in_=ps[0:NJ2, :]
                    )
                else:
                    nc.scalar.copy(
                        out=obs[g][:, ls * C:(ls + 1) * C], in_=ps[0:NJ2, :]
                    )
                if i in grp_end:
                    g = grp_end[i]
                    out_engs[g % len(out_engs)].dma_start(
                        out=outd[:, int(gs0[g]) * C:int(gs0[g + 1]) * C],
                        in_=obs[g][:],
                    )
                a += L[i]
    nc.compile()
    return nc, key


# --------------------------------------------------------------------------
# entry point
# --------------------------------------------------------------------------
def kernel(input, rois, offset):
    from concourse.bass_utils import run_bass_kernel_spmd

    input = np.asarray(input, dtype=f32)
    d, pts, px, pairs, cores, L, T, nslot = _plan(rois, offset)

    in_groups = _split_groups(T)
    out_grps = _out_groups(nslot)
    key = (T, tuple(int(x) for x in L), in_groups, out_grps, nslot)
    if key not in _prog_cache:
        _prog_cache[key] = _build_program(key)
    nc, _ = _prog_cache[key]

    # channel-last fp8 feature map, flat pixel index
    fcl8 = np.ascontiguousarray(
        input.transpose(0, 2, 3, 1).astype(E3M4)
    ).reshape(B * H * W, C)

    a_starts = np.concatenate([[0], np.cumsum(L)]).astype(int)
    R = int(a_starts[-1])
    batch = d["batch"]

    in_maps = []
    for k in range(N_CORES):
        stream = np.zeros((128, T * SW), dtype=E3M4)
        srows = np.zeros((T * 128, SW), dtype=E3M4)  # row-major scratch
        for i, p in enumerate(cores[k]):
            ra, rb = pairs[p]
            members = [(ra, 0)] + ([(rb, NJ)] if rb >= 0 else [])
            pset = sorted(px[ra] | (px[rb] if rb >= 0 else frozenset()))
            if not pset:
                continue
            r0 = int(a_starts[i])
            # pixels (pset entries are (img, y, x) triples)
            bs = np.array([t[0] for t in pset])
            ys = np.array([t[1] for t in pset])
            xs = np.array([t[2] for t in pset])
            srows[r0:r0 + len(pset), 0:C] = fcl8[bs * (H * W) + ys * W + xs]
            # A-weights (accumulate taps in f64, then quantize once)
            acc = np.zeros((len(pset), NJ2), f64)
            pos = {t: r for r, t in enumerate(pset)}
            for n, cb in members:
                if pts[n] is None:
                    continue
                bn = int(batch[n])
                yy, xx, jc, ww = pts[n]
                lp = np.array([pos[(bn, y, x)]
                               for y, x in zip(yy.tolist(), xx.tolist())])
                np.add.at(acc, (lp, jc + cb), ww * A_SCALE)
            srows[r0:r0 + len(pset), C:SW] = acc.astype(f32).astype(E3M4)
        # [T*128, SW] -> [128, T, SW] -> [128, T*SW]
        stream[:] = srows.reshape(T, 128, SW).transpose(1, 0, 2).reshape(128, T * SW)
        in_maps.append({"strm": stream})

    res = run_bass_kernel_spmd(nc, in_maps, core_ids=list(range(N_CORES)))

    out_full = np.empty((N_ROIS, C, P, P), f32)
    inv = f32(1.0 / A_SCALE)
    for k in range(N_CORES):
        arr = res.results[k]["out"].astype(f32).reshape(NJ2, nslot, C)
        for i, p in enumerate(cores[k]):
            ra, rb = pairs[p]
            out_full[ra] = (arr[0:NJ, i, :] * inv).T.reshape(C, P, P)
            if rb >= 0:
                out_full[rb] = (arr[NJ:NJ2, i, :] * inv).T.reshape(C, P, P)
    return out_full


# revision 30
# speedup vs baseline: 1.2410x; 1.0413x over previous
"""DCNv2 deformable PS-RoI pooling on 8 Trainium2 NeuronCores — v3.

Strategy (fp8 union-pair stream, 32-row windows):
  * Host replicates the reference coordinate math exactly (f32) and folds
    bilinear weights, validity masking and 1/count into per-roi sparse weights.
  * Rois are paired per image minimizing ceil32(|union pixel set|); exact
    pixel sets (not bounding boxes) are streamed.  128 pairs -> 16 per core.
  * Pairs are rank-matched across cores: slot i spans rows
    [a_i, a_i + L_i) of a shared row space, L_i = max over cores of the
    rank-i pair's 32-aligned pixel count.  Row windows are program
    constants, so the SPMD matmul schedule (partition-sliced lhsT/rhs) is
    identical on every core while per-core pixel data differs.
  * Stream dtype float8_e3m4 for BOTH pixels and A-weights (A pre-scaled
    x8; host divides the output back).  Measured l2 rel err ~1.4e-2.
    Per stream row: 256 px bytes + 98 weight bytes = 354 B (vs 708 f16).
  * One matmul per (window, chunk) piece: out = psum[98, 256] accumulated
    over the window's chunks; partition-offset matmuls slice the window's
    rows within each 128-row chunk (legal bases {0,32,64,96}).
  * PE p-state warm-up: a run of dummy matmuls at program start ramps the
    tensor engine to full clock before real data arrives.
  * Drains alternate DVE/Act; output DMAs ride the Pool SWDGE queue so the
    HWDGE device only serves input loads.
"""
import numpy as np
import ml_dtypes

f32 = np.float32
f64 = np.float64
E3M4 = ml_dtypes.float8_e3m4

B, C, H, W = 8, 256, 64, 64
N_ROIS, P, S = 256, 7, 4
PART = 7
NJ = P * P              # 49
NJ2 = 2 * NJ            # 98 columns per pair
SCALE = f32(1.0 / 16.0)
TRANS_STD = f32(0.1)
N_CORES = 8
A_SCALE = 8.0           # weights pre-scaled x8 (max w <= 1, e3m4 max 15.5)
SW = C + NJ2            # 354 bytes per stream row (f8)
N_PSUM_TAGS = 7
WARMUP_MM = 2

_prog_cache = {}


# --------------------------------------------------------------------------
# host math: exact f32 replication of the reference coordinate computation
# --------------------------------------------------------------------------
def _roi_sampling_data(rois, offset):
    rois = np.asarray(rois, dtype=f32)
    offset = np.asarray(offset, dtype=f32)
    batch = rois[:, 0].astype(np.int32)

    roi_sw = np.round(rois[:, 1]) * SCALE - f32(0.5)
    roi_sh = np.round(rois[:, 2]) * SCALE - f32(0.5)
    roi_ew = (np.round(rois[:, 3]) + f32(1.0)) * SCALE - f32(0.5)
    roi_eh = (np.round(rois[:, 4]) + f32(1.0)) * SCALE - f32(0.5)
    roi_w = np.maximum(roi_ew - roi_sw, f32(0.1))
    roi_h = np.maximum(roi_eh - roi_sh, f32(0.1))
    bin_w = roi_w / f32(P)
    bin_h = roi_h / f32(P)
    sub_w = bin_w / f32(S)
    sub_h = bin_h / f32(S)

    ph = np.arange(P, dtype=np.int32)
    pw = np.arange(P, dtype=np.int32)
    part_h = np.clip(
        np.floor(ph.astype(f32) / f32(P) * f32(PART)).astype(np.int32), 0, PART - 1
    )
    part_w = np.clip(
        np.floor(pw.astype(f32) / f32(P) * f32(PART)).astype(np.int32), 0, PART - 1
    )

    tx = offset[:, 0][:, part_h[:, None], part_w[None, :]] * TRANS_STD  # (N,7,7)
    ty = offset[:, 1][:, part_h[:, None], part_w[None, :]] * TRANS_STD

    wstart = (
        pw.astype(f32)[None, None, :] * bin_w[:, None, None]
        + roi_sw[:, None, None]
        + tx * roi_w[:, None, None]
    )
    hstart = (
        ph.astype(f32)[None, :, None] * bin_h[:, None, None]
        + roi_sh[:, None, None]
        + ty * roi_h[:, None, None]
    )

    iw = np.arange(S, dtype=f32)
    ih = np.arange(S, dtype=f32)
    wpos = (
        wstart[:, :, :, None, None]
        + iw[None, None, None, None, :] * sub_w[:, None, None, None, None]
    )
    hpos = (
        hstart[:, :, :, None, None]
        + ih[None, None, None, :, None] * sub_h[:, None, None, None, None]
    )

    valid = (
        (wpos >= f32(-0.5)) & (wpos <= f32(W) - f32(0.5))
        & (hpos >= f32(-0.5)) & (hpos <= f32(H) - f32(0.5))
    )
    wc = np.clip(wpos, f32(0.0), f32(W - 1.0))
    hc = np.clip(hpos, f32(0.0), f32(H - 1.0))

    x0 = np.floor(wc).astype(np.int32)
    x1 = np.ceil(wc).astype(np.int32)
    y0 = np.floor(hc).astype(np.int32)
    y1 = np.ceil(hc).astype(np.int32)
    dx = (wc - np.floor(wc)).astype(f64)
    dy = (hc - np.floor(hc)).astype(f64)

    cnt = valid.sum(axis=(3, 4)).astype(f32)  # (N,7,7)
    coef = np.where(cnt > 0, 1.0 / np.maximum(cnt, f32(1.0)).astype(f64), 0.0)

    w00 = (1.0 - dx) * (1.0 - dy)
    w01 = dx * (1.0 - dy)
    w10 = (1.0 - dx) * dy
    w11 = dx * dy

    return dict(
        batch=batch, valid=valid, x0=x0, x1=x1, y0=y0, y1=y1,
        w00=w00, w01=w01, w10=w10, w11=w11, coef=coef,
    )


def _roi_points(d, n):
    """All (y, x, j, w) bilinear contributions of roi n, valid-masked."""
    full = (P, P, S, S)
    v = d["valid"][n]
    if not v.any():
        return None
    jj = np.broadcast_to(
        np.arange(NJ, dtype=np.int64).reshape(P, P, 1, 1), full
    )[v]
    xs0 = np.broadcast_to(d["x0"][n], full)[v]
    xs1 = np.broadcast_to(d["x1"][n], full)[v]
    ys0 = np.broadcast_to(d["y0"][n], full)[v]
    ys1 = np.broadcast_to(d["y1"][n], full)[v]
    cf = np.broadcast_to(d["coef"][n][:, :, None, None], full)[v]
    yy = np.concatenate([ys0, ys0, ys1, ys1])
    xx = np.concatenate([xs0, xs1, xs0, xs1])
    jc = np.concatenate([jj, jj, jj, jj])
    ww = np.concatenate([
        np.broadcast_to(d["w00"][n], full)[v] * cf,
        np.broadcast_to(d["w01"][n], full)[v] * cf,
        np.broadcast_to(d["w10"][n], full)[v] * cf,
        np.broadcast_to(d["w11"][n], full)[v] * cf,
    ])
    return yy, xx, jc, ww


def _ceil64(x):
    """Window lengths are 64-aligned: matmul partition bases are then always
    0 or 64, the only tile positions that proved safe to mix inside one PE
    accumulation group on hardware (base-32 pieces crash the neuron
    runtime when mixed with 64/128-row pieces)."""
    return max((int(x) + 63) // 64 * 64, 64)


def _plan(rois, offset):
    """Pair rois, deal pairs to cores, fix rank-max 32-aligned row windows."""
    d = _roi_sampling_data(rois, offset)
    pts = [_roi_points(d, n) for n in range(N_ROIS)]
    batch = d["batch"]

    # pixel key includes the image index so cross-image leftover pairs keep
    # their pixels distinct
    px = []
    for n in range(N_ROIS):
        if pts[n] is None:
            px.append(frozenset())
        else:
            bn = int(batch[n])
            px.append(frozenset(
                (bn, y, x)
                for y, x in zip(pts[n][0].tolist(), pts[n][1].tolist())
            ))

    pairs = []
    leftover = []
    for b in range(B):
        idx = sorted(
            [n for n in range(N_ROIS) if batch[n] == b],
            key=lambda n: (-len(px[n]), n),
        )
        while len(idx) >= 2:
            a = idx.pop(0)
            best = min(
                range(len(idx)),
                key=lambda j: (
                    _ceil64(len(px[a] | px[idx[j]])),
                    -len(px[a] & px[idx[j]]),
                    idx[j],
                ),
            )
            pairs.append((a, idx.pop(best)))
        leftover += idx
    leftover.sort()
    while len(leftover) >= 2:
        pairs.append((leftover.pop(), leftover.pop()))
    if leftover:
        pairs.append((leftover.pop(), -1))

    plen = np.array([len(px[a] | (px[b] if b >= 0 else frozenset()))
                     for a, b in pairs])

    order = np.argsort(-plen, kind="stable")
    cores = [[] for _ in range(N_CORES)]
    tot = [0] * N_CORES
    for p in order:
        k = int(np.argmin(tot))
        cores[k].append(int(p))
        tot[k] += _ceil64(plen[p])
    # ascending window order: small windows early (feed early output
    # groups), then one SMALL window moved to the very end so only one
    # matmul piece and one drain trail the final input transfer
    for k in range(N_CORES):
        lst = cores[k][::-1]
        cores[k] = lst[1:] + [lst[0]]

    nslot = max(len(c) for c in cores)
    L = [
        max((_ceil64(plen[c[i]]) if i < len(c) else 64) for c in cores)
        for i in range(nslot)
    ]
    # A window with start==64 (mod 128) and L==128 emits the piece sequence
    # [64:128] -> [0:64]: same PE row-size bucket at a different tile
    # position inside one accumulation group, which crashes the neuron
    # runtime.  Pad the previous window by 64 rows to shift the start.
    a = 0
    for i in range(nslot):
        if a % 128 == 64 and L[i] == 128:
            L[i - 1] += 64
            a += 64
        a += L[i]
    R = int(sum(L))
    T = (R + 127) // 128
    return d, pts, px, pairs, cores, L, T, nslot


def _split_groups(T):
    """Input-load chunk groups.  Few groups: each DMA instruction costs
    ~1.2us of SEQ+HWDGE issue overhead, so k is kept small; the first group
    is small for a fast pipeline start."""
    assert T >= 12
    mid = T - 11
    m1 = (mid + 1) // 2
    return (5, m1, mid - m1, 5, 1)


def _out_groups(nslot):
    """Output-store slot groups; final group stays small for a short tail."""
    if nslot <= 3:
        return tuple([nslot - 1, 1]) if nslot > 1 else (1,)
    h1 = max(nslot - 9, 1)
    rest = nslot - h1 - 1
    h2 = (rest + 1) // 2
    h3 = rest - h2
    return tuple(x for x in (h1, h2, h3, 1) if x > 0)


def _window_pieces(a, bnd):
    """Legal matmul partition slices for row window [a, bnd) per 128-chunk.

    Matmul base partitions are restricted to {0,32,64,96} with size buckets
    (base 32 -> size <= 32), so a [32,128) slice splits in two.
    """
    out = []
    for c in range(a // 128, (bnd + 127) // 128):
        r0 = max(a, c * 128) - c * 128
        r1 = min(bnd, (c + 1) * 128) - c * 128
        assert r0 in (0, 64), (a, bnd, c, r0)
        out.append((c, r0, r1))
    for (c1, p0, p1), (c2, q0, q1) in zip(out, out[1:]):
        b1, b2 = (128 if p1 - p0 > 64 else 64), (128 if q1 - q0 > 64 else 64)
        assert not (b1 == b2 and p0 != q0), ("unsafe PE tile seq", out)
    return out


# --------------------------------------------------------------------------
# device program
# --------------------------------------------------------------------------
def _build_program(key):
    import concourse.bacc as bacc
    import concourse.mybir as mybir
    from concourse.tile import TileContext

    T, L, in_groups, out_grps, nslot = key
    L = list(L)

    nc = bacc.Bacc("TRN2", num_devices=N_CORES)
    dt = mybir.dt
    strm = nc.dram_tensor("strm", [128, T * SW], dt.float8e3, kind="ExternalInput")
    outd = nc.dram_tensor("out", [NJ2, nslot * C], dt.float16, kind="ExternalOutput")

    # chunk -> (group idx, local chunk offset)
    c2g = {}
    c0 = 0
    for g, nch in enumerate(in_groups):
        for c in range(c0, c0 + nch):
            c2g[c] = (g, c - c0)
        c0 += nch

    with TileContext(nc) as tc:
        with (
            tc.tile_pool(name="main", bufs=1) as mp,
            tc.tile_pool(name="psum", bufs=1, space="PSUM") as pp,
        ):
            # PE p-state warm-up: pe_busy_start is pinned by the FIRST PE
            # instruction and never resets, so two tiny matmuls as early as
            # possible make everything ~3us later run at full clock.  The
            # memset rides the Pool engine, which is free right after the
            # preamble.
            zt = mp.tile([128, 2], dt.float8e3, tag="zt")
            nc.gpsimd.memset(zt[:], 0.0)
            # warm-up psum lives in a scratch corner that window 0 (start=True)
            # later overwrites -- all 8 PSUM banks stay available for windows
            wps = pp.tile([128, 2 * C], dt.float32, tag="ps0")
            for _ in range(WARMUP_MM):
                nc.tensor.matmul(
                    out=wps[0:1, 0:1], lhsT=zt[:, 0:1], rhs=zt[:, 1:2],
                    start=True, stop=True,
                )

            # input loads alternate the two HWDGE queues so SEQ issue
            # (~650ns per DMA) does not serialize ahead of the transfers
            in_engs = [nc.sync, nc.scalar]
            st = []
            c0 = 0
            for g, nch in enumerate(in_groups):
                t_g = mp.tile([128, nch * SW], dt.float8e3, tag=f"strm{g}")
                in_engs[g % 2].dma_start(
                    out=t_g[:], in_=strm[:, c0 * SW:(c0 + nch) * SW]
                )
                st.append(t_g)
                c0 += nch

            # out buffers per store group
            obs = []
            s0 = 0
            slot2grp = {}
            for g, ns in enumerate(out_grps):
                ob = mp.tile([NJ2, ns * C], dt.float16, tag=f"ob{g}")
                obs.append(ob)
                for s in range(s0, s0 + ns):
                    slot2grp[s] = (g, s - s0)
                s0 += ns

            # matmul schedule: fixed row windows, partition-sliced pieces.
            # Each output store is emitted right after its group's last drain
            # so SEQ-queue ordering never couples it to later windows.
            # PSUM: two windows share one [128, 512] f32 bank tile; 8 banks
            # hold all 16 windows, so matmuls NEVER wait on drains.
            out_engs = [nc.sync, nc.gpsimd, nc.sync, nc.gpsimd]
            grp_end = {}
            s0 = 0
            for g, ns in enumerate(out_grps):
                grp_end[s0 + ns - 1] = g
                s0 += ns
            gs0 = np.concatenate([[0], np.cumsum(out_grps)]).astype(int)
            assert nslot <= 16
            pstiles = [
                pp.tile([128, 2 * C], dt.float32, tag=f"ps{q}", name=f"ps{q}")
                for q in range((nslot + 1) // 2)
            ]
            a = 0
            for i in range(nslot):
                pieces = _window_pieces(a, a + L[i])
                pt = pstiles[i // 2]
                pc0 = (i % 2) * C
                for k, (c, r0, r1) in enumerate(pieces):
                    g, lc = c2g[c]
                    t_g = st[g]
                    col = lc * SW
                    nc.tensor.matmul(
                        out=pt[0:NJ2, pc0:pc0 + C],
                        lhsT=t_g[r0:r1, col + C:col + SW],
                        rhs=t_g[r0:r1, col:col + C],
                        start=(k == 0),
                        stop=(k == len(pieces) - 1),
                    )
                g, ls = slot2grp[i]
                # GPSIMD cannot read PSUM, so drains alternate DVE/Act
                if i % 2 == 0:
                    nc.vector.tensor_copy(
                        out=obs[g][:, ls * C:(ls + 1) * C],
                        in_=pt[0:NJ2, pc0:pc0 + C],
                    )
                else:
                    nc.scalar.copy(
                        out=obs[g][:, ls * C:(ls + 1) * C],
                        in_=pt[0:NJ2, pc0:pc0 + C],
                    )
                if i in grp_end:
                    g = grp_end[i]
                    out_engs[g % len(out_engs)].dma_start(
                        out=outd[:, int(gs0[g]) * C:int(gs0[g + 1]) * C],
                        in_=obs[g][:],
                    )
                a += L[i]
    nc.compile()
    return nc, key


# --------------------------------------------------------------------------
# entry point
# --------------------------------------------------------------------------
def kernel(input, rois, offset):
    from concourse.bass_utils import run_bass_kernel_spmd

    input = np.asarray(input, dtype=f32)
    d, pts, px, pairs, cores, L, T, nslot = _plan(rois, offset)

    in_groups = _split_groups(T)
    out_grps = _out_groups(nslot)
    key = (T, tuple(int(x) for x in L), in_groups, out_grps, nslot)
    if key not in _prog_cache:
        _prog_cache[key] = _build_program(key)
    nc, _ = _prog_cache[key]

    # channel-last fp8 feature map, flat pixel index
    fcl8 = np.ascontiguousarray(
        input.transpose(0, 2, 3, 1).astype(E3M4)
    ).reshape(B * H * W, C)

    a_starts = np.concatenate([[0], np.cumsum(L)]).astype(int)
    R = int(a_starts[-1])
    batch = d["batch"]

    in_maps = []
    for k in range(N_CORES):
        stream = np.zeros((128, T * SW), dtype=E3M4)
        srows = np.zeros((T * 128, SW), dtype=E3M4)  # row-major scratch
        for i, p in enumerate(cores[k]):
            ra, rb = pairs[p]
            members = [(ra, 0)] + ([(rb, NJ)] if rb >= 0 else [])
            pset = sorted(px[ra] | (px[rb] if rb >= 0 else frozenset()))
            if not pset:
                continue
            r0 = int(a_starts[i])
            # pixels (pset entries are (img, y, x) triples)
            bs = np.array([t[0] for t in pset])
            ys = np.array([t[1] for t in pset])
            xs = np.array([t[2] for t in pset])
            srows[r0:r0 + len(pset), 0:C] = fcl8[bs * (H * W) + ys * W + xs]
            # A-weights (accumulate taps in f64, then quantize once)
            acc = np.zeros((len(pset), NJ2), f64)
            pos = {t: r for r, t in enumerate(pset)}
            for n, cb in members:
                if pts[n] is None:
                    continue
                bn = int(batch[n])
                yy, xx, jc, ww = pts[n]
                lp = np.array([pos[(bn, y, x)]
                               for y, x in zip(yy.tolist(), xx.tolist())])
                np.add.at(acc, (lp, jc + cb), ww * A_SCALE)
            srows[r0:r0 + len(pset), C:SW] = acc.astype(f32).astype(E3M4)
        # [T*128, SW] -> [128, T, SW] -> [128, T*SW]
        stream[:] = srows.reshape(T, 128, SW).transpose(1, 0, 2).reshape(128, T * SW)
        in_maps.append({"strm": stream})

    res = run_bass_kernel_spmd(nc, in_maps, core_ids=list(range(N_CORES)))

    out_full = np.empty((N_ROIS, C, P, P), f32)
    inv = f32(1.0 / A_SCALE)
    for k in range(N_CORES):
        arr = res.results[k]["out"].astype(f32).reshape(NJ2, nslot, C)
        for i, p in enumerate(cores[k]):
            ra, rb = pairs[p]
            out_full[ra] = (arr[0:NJ, i, :] * inv).T.reshape(C, P, P)
            if rb >= 0:
                out_full[rb] = (arr[NJ:NJ2, i, :] * inv).T.reshape(C, P, P)
    return out_full
